# revision 1
# baseline (speedup 1.0000x reference)
"""Radius-graph adjacency mask (radius_graph r=3, loop=True) on 8 TRN2 NeuronCores.

Strategy
--------
mask[i, j] = (||p_i - p_j||^2 <= R2)  for pos [8192, 3].

val(i, j) = (R2 + eps) - d2(i, j) is computed as a single small-K matmul:
    val = sum_r q_rows[r, i] * k_rows[r, j]
where the q/k rows hold 3-way bf16 splits of the augmented query/key vectors
(2x, sq terms), so the bf16 TensorE matmul (1 cycle/row) reproduces the fp32
value to ~24-bit accuracy.  PSUM holds val; mask = (val >= 0) via VectorE
is_ge / ScalarE Sign (both engines split the PSUM-read load), written as int8
and DMA'd out.

Sharding: rows data-parallel across 8 cores (1024 query rows each).  Atoms are
z-sorted; in symmetric mode each 128-query block computes only keys at sorted
index >= its own start inside the z-window (all forward |z_i - z_j| <= 3
neighbors) — a W~1024 slab — and the host mirrors the lower triangle.  Each
core holds ONE shared key window [128*8c, 128*8c + WC); block b reads columns
[128b, 128b + W) of it.  The host scatters the slabs into the full mask.
"""

from contextlib import ExitStack

import ml_dtypes
import numpy as np

import concourse.mybir as mybir
from concourse import bacc
from concourse.bass_utils import run_bass_kernel_spmd

N = 8192
R2 = 9.0
RADIUS = 3.0
EPS = 1e-5
NCORES = 8
P = 128
KP = 32                       # padded contraction rows (30 used)
BLOCKS = (N // NCORES) // P   # 8 query blocks of 128 rows per core
BF16 = ml_dtypes.bfloat16

def _bf16_split3(x):
    """Split f64 array into 3 bf16 components summing to ~24-bit accuracy."""
    b0 = x.astype(BF16)
    r1 = x - b0.astype(np.float64)
    b1 = r1.astype(BF16)
    r2 = r1 - b1.astype(np.float64)
    b2 = r2.astype(BF16)
    return b0.astype(np.float64), b1.astype(np.float64), b2.astype(np.float64)


def _build_rows(ps):
    """Build the KP-row augmented query/key matrices (f64 holding bf16 values).

    val = sum_r q_rows[r, i] * k_rows[r, j] = (R2 + EPS) - d2(i, j)
    """
    n = ps.shape[0]
    A = 2.0 * ps.T                      # (3, n) query-side coefficient
    B = ps.T                            # (3, n) key-side
    S = (R2 + EPS) - (ps * ps).sum(1)   # query-side constant term
    T = -(ps * ps).sum(1)               # key-side constant term
    ones = np.ones(n)

    rows_q, rows_k = [], []
    for c in range(3):
        Asp = _bf16_split3(A[c])
        Bsp = _bf16_split3(B[c])
        # all split-product terms above ~2^-32 relative (drop (2,2) only)
        for u, v in [(0, 0), (0, 1), (1, 0), (1, 1), (0, 2), (2, 0), (1, 2), (2, 1)]:
            rows_q.append(Asp[u])
            rows_k.append(Bsp[v])
    for s in _bf16_split3(S):
        rows_q.append(s)
        rows_k.append(ones)
    for t in _bf16_split3(T):
        rows_q.append(ones)
        rows_k.append(t)

    q = np.zeros((KP, n))
    k = np.zeros((KP, n))
    q[: len(rows_q)] = np.stack(rows_q)
    k[: len(rows_k)] = np.stack(rows_k)
    return q, k



def _psum_slot(b, engine_map=True):
    """engine_map: ACT (even b) slots 0/1 (banks 0-3); DVE (odd b) slots 2/3."""
    return (b % 2) * 2 + (b // 2) % 2 if engine_map else b % 4

def _build_graph_shared_raw(W, WC, final_wait=False, k2_sync=True, psum_engine_map=True,
                            last_split=False, swap_parity=False):
    """Raw Block version of the symmetric shared-window graph.

    Manual engine streams + semaphores (no TileContext): saves the Tile
    entry/exit drain + barrier + sem-clear machinery (~4us of exec window).

    Engine roles: sync = q + k-tail DMA; scalar = k-head DMA + Sign x4;
    vector = is_ge x4; tensor = matmuls; gpsimd = output DMAs.

    When NT == 2 the q tensor is packed: each block only needs row groups
    {2b%4, (2b+1)%4}, so even blocks live at partitions 0..63, odd at 64..127,
    slot b//2 — halving the q transfer.
    """
    assert W % 64 == 0 and W <= 2048
    NT = -(-W // 512)
    q_packed = NT == 2
    QS = BLOCKS // 2 if q_packed else BLOCKS

    def eng_of(b):  # which engine thresholds block b
        return "act" if (b % 2 == 0) != swap_parity else "dve"

    nc = bacc.Bacc("TRN2", target_bir_lowering=False)
    q_ext = nc.declare_dram_parameter("q", [P, QS, P], mybir.dt.bfloat16, isOutput=False)
    k_ext = nc.declare_dram_parameter("k", [P, WC], mybir.dt.bfloat16, isOutput=False)
    out_ext = nc.declare_dram_parameter("out", [BLOCKS, P, W], mybir.dt.int8, isOutput=True)

    # count of same-engine thresholds among blocks 0..b inclusive
    def eng_count(b, eng):
        return sum(1 for x in range(b + 1) if eng_of(x) == eng)

    with ExitStack() as ctx:
        qsem = ctx.enter_context(nc.semaphore("qsem"))
        ksem = ctx.enter_context(nc.semaphore("ksem"))
        ksem1b = ctx.enter_context(nc.semaphore("ksem1b"))
        ksem2 = ctx.enter_context(nc.semaphore("ksem2"))
        ksem2b = ctx.enter_context(nc.semaphore("ksem2b"))
        pe_sem = ctx.enter_context(nc.semaphore("pe_sem"))
        act_sem = ctx.enter_context(nc.semaphore("act_sem"))
        dve_sem = ctx.enter_context(nc.semaphore("dve_sem"))
        osem = ctx.enter_context(nc.semaphore("osem"))
        wsem = ctx.enter_context(nc.semaphore("wsem"))
        scratch = ctx.enter_context(nc.sbuf_tensor("scratch", [P, 640], mybir.dt.bfloat16))
        q_sb = ctx.enter_context(nc.sbuf_tensor("q_sb", [P, QS, P], mybir.dt.bfloat16))
        k_sb = ctx.enter_context(nc.sbuf_tensor("k_sb", [P, WC], mybir.dt.bfloat16))
        masks = [
            ctx.enter_context(nc.sbuf_tensor(f"m{i}", [P, 2, W], mybir.dt.int8))
            for i in range(BLOCKS // 2)
        ]
        psums = [
            ctx.enter_context(nc.psum_tensor(f"ps{i}", [P, W], mybir.dt.float32))
            for i in range(4)
        ]

        SPLIT_B = BLOCKS - 1                  # last block: split across engines
        # balance ACT (4 full blocks + H cols) vs DVE (3 full + W-H cols):
        # 5*oA + (4W+H)*eA = 4*oD + (4W-H)*eD with per-op overheads/rates
        _h = (4 * 125 - 5 * 143 + 4 * W * (1.042 - 0.833)) / (0.833 + 1.042)
        HALF = int(max(64, min(W - 64, round(_h / 64) * 64)))

        def _thresh(engine, b, lo=0, hi=None):
            hi = W if hi is None else hi
            slot = _psum_slot(b, psum_engine_map)
            if engine.engine == mybir.EngineType.Activation:
                return engine.activation(
                    masks[b // 2][:, b % 2, lo:hi], psums[slot][:, lo:hi],
                    mybir.ActivationFunctionType.Sign,
                ).then_inc(act_sem, 1)
            return engine.tensor_scalar(
                masks[b // 2][:, b % 2, lo:hi], psums[slot][:, lo:hi],
                0.0, None, mybir.AluOpType.is_ge,
            ).then_inc(dve_sem, 1)

        with nc.Block() as block:

            MID = W + max(64, ((WC - W) // 2) // 64 * 64) if WC > W else WC
            # key pieces: [start, end, sem) — MMs wait per piece on first use.
            # One [0:W] head so block 0's two matmul tiles (distinct PE row
            # groups) become ready together and run concurrently.
            pieces = [(0, W, ksem)]
            if WC > W:
                pieces.append((W, MID, ksem2))
                if MID < WC:
                    pieces.append((MID, WC, ksem2b))

            @block.sync
            def _(sync):
                sync.dma_start(out=q_sb[:], in_=q_ext[:]).then_inc(qsem, 16)
                if WC > W and MID < WC:
                    sync.dma_start(out=k_sb[:, MID:], in_=k_ext[:, MID:]).then_inc(ksem2b, 16)

            @block.scalar
            def _(scalar):
                scalar.dma_start(out=k_sb[:, :W], in_=k_ext[:, :W]).then_inc(ksem, 16)
                for b in range(BLOCKS):
                    if b == SPLIT_B:
                        scalar.wait_ge(pe_sem, b + 1)
                        _thresh(scalar, b, 0, HALF)
                    elif eng_of(b) == "act":
                        scalar.wait_ge(pe_sem, b + 1)
                        _thresh(scalar, b)

            @block.vector
            def _(vector):
                vector.memset(scratch[:], 0).then_inc(wsem, 1)
                for b in range(BLOCKS):
                    if b == SPLIT_B:
                        vector.wait_ge(pe_sem, b + 1)
                        _thresh(vector, b, HALF, W)
                    elif eng_of(b) == "dve":
                        vector.wait_ge(pe_sem, b + 1)
                        _thresh(vector, b)

            @block.tensor
            def _(tensor):
                # HAM warmup: ~3us of dummy matmuls on zeroed scratch while
                # the input DMAs are in flight, so real matmuls run at 2.4 GHz.
                # Results land in ps0 and are overwritten (start=True) later.
                tensor.wait_ge(wsem, 1)
                for w in range(5):
                    g = 2 + w % 2          # groups 2/3: block 0 uses 0/1
                    # psums[3] (its owner b3 shares row groups 2/3 so it
                    # serializes after); per-group DISTINCT banks: concurrent
                    # PE writes to one PSUM bank are a fatal collision
                    wn = 512 if g == 2 else min(448, W - 512)
                    wo = 0 if g == 2 else 512
                    tensor.matmul(
                        psums[3][:, wo : wo + wn],
                        lhsT=scratch[32 * g : 32 * (g + 1), :128],
                        rhs=scratch[32 * g : 32 * (g + 1), 128 : 128 + wn],
                        start=True,
                        stop=True,
                        tile_position=(32 * g, 0),
                    )
                tensor.wait_ge(qsem, 16)
                tensor.wait_ge(ksem, 16)
                waited = {id(ksem)}
                for b in range(BLOCKS):
                    if b >= 4:  # psum slot reuse: wait for block b-4's threshold
                        prev = b - 4
                        if eng_of(prev) == "act":
                            tensor.wait_ge(act_sem, eng_count(prev, "act"))
                        else:
                            tensor.wait_ge(dve_sem, eng_count(prev, "dve"))

                    for t in range(NT):
                        g = (NT * b + t) % 4
                        col = P * b + 512 * t
                        nn = min(512, W - 512 * t)
                        for p0, p1, sem in pieces:
                            if id(sem) not in waited and col + nn > p0 and col < p1:
                                tensor.wait_ge(sem, 16)
                                waited.add(id(sem))
                        mm = tensor.matmul(
                            psums[_psum_slot(b, psum_engine_map)][:, 512 * t : 512 * t + nn],
                            lhsT=q_sb[32 * g : 32 * (g + 1), b // 2 if q_packed else b, :],
                            rhs=k_sb[32 * g : 32 * (g + 1), col : col + nn],
                            start=True,
                            stop=True,
                            tile_position=(32 * g, 0),
                        )
                        if t == NT - 1:
                            mm.then_inc(pe_sem, 1)

            @block.gpsimd
            def _(gpsimd):
                if WC > W:
                    gpsimd.dma_start(out=k_sb[:, W:MID], in_=k_ext[:, W:MID]).then_inc(ksem2, 16)
                last = BLOCKS // 2 - 1
                ndma = 0
                for i in range(last):
                    gpsimd.wait_ge(act_sem, i + 1)
                    gpsimd.wait_ge(dve_sem, i + 1)
                    gpsimd.dma_start(
                        out=out_ext[2 * i : 2 * i + 2, :, :].rearrange("b p w -> p b w"),
                        in_=masks[i][:],
                    ).then_inc(osem, 16)
                    ndma += 1
                if last_split:
                    s0 = act_sem if eng_of(2 * last) == "act" else dve_sem
                    s1 = act_sem if eng_of(2 * last + 1) == "act" else dve_sem
                    gpsimd.wait_ge(s0, last + 1)
                    gpsimd.dma_start(
                        out=out_ext[2 * last : 2 * last + 1, :, :].rearrange("b p w -> p b w"),
                        in_=masks[last][:, :1],
                    ).then_inc(osem, 16)
                    gpsimd.wait_ge(s1, last + 1)
                    gpsimd.dma_start(
                        out=out_ext[2 * last + 1 : 2 * last + 2, :, :].rearrange("b p w -> p b w"),
                        in_=masks[last][:, 1:],
                    ).then_inc(osem, 16)
                    ndma += 2
                else:
                    # block 2*last is a normal single-engine threshold; block
                    # 2*last+1 (SPLIT_B) contributes one inc on EACH engine
                    gpsimd.wait_ge(act_sem, eng_count(BLOCKS - 2, "act") + 1)
                    gpsimd.wait_ge(dve_sem, eng_count(BLOCKS - 2, "dve") + 1)
                    gpsimd.dma_start(
                        out=out_ext[2 * last : 2 * last + 2, :, :].rearrange("b p w -> p b w"),
                        in_=masks[last][:],
                    ).then_inc(osem, 16)
                    ndma += 1
                if final_wait:
                    gpsimd.wait_ge(osem, 16 * ndma)

    nc.compile()
    return nc


def _prepare(pos):
    """Host prep: pick the sort axis with the tightest symmetric window, build
    per-core in_maps.  Returns None when no axis gives a device-sized window
    (degenerate clustered input) -- caller falls back to host computation."""
    posf = np.asarray(pos, dtype=np.float64)
    nblocks = N // P

    # recenter: d2 is translation-invariant, but smaller |coords| shrink the
    # fp32 cancellation error in sq_i + sq_j - 2 x.y by ~4x
    posf = posf - (posf.min(0) + posf.max(0)) / 2.0

    best = None
    for axis in range(3):
        order = np.argsort(posf[:, axis], kind="stable")
        z = posf[order][:, axis]
        zb = z.reshape(nblocks, P)
        ihi = np.searchsorted(z, zb.max(1) + RADIUS, side="right")
        w_sym = int((ihi - np.arange(nblocks, dtype=np.int64) * P).max())
        if best is None or w_sym < best[0]:
            best = (w_sym, order)
    w_sym, order = best
    if w_sym > 2048:
        return None

    ps = posf[order]
    W = max(512, -(-w_sym // 64) * 64)
    WC = P * (BLOCKS - 1) + W
    q_packed = -(-W // 512) == 2
    qrows, krows = _build_rows(ps)
    q16 = qrows.astype(BF16)
    # pad key tail with far-away dummies (mask always 0 there)
    k16 = np.zeros((KP, N + WC), dtype=BF16)
    k16[:, :N] = krows.astype(BF16)
    k16[KP - 3, N:] = -1e9              # T0 row: val = S_i - 1e9 < 0
    in_maps = []
    for c in range(NCORES):
        if q_packed:
            # block b lives at row groups {2b%4, (2b+1)%4}, slot b//2
            qc = np.zeros((P, BLOCKS // 2, P), dtype=BF16)
            for b in range(BLOCKS):
                g = c * BLOCKS + b
                qb = q16[:, g * P : (g + 1) * P]
                base = 0 if b % 2 == 0 else 64
                qc[base : base + 64, b // 2, :] = np.tile(qb, (2, 1))
        else:
            qc = np.zeros((P, BLOCKS, P), dtype=BF16)
            for b in range(BLOCKS):
                g = c * BLOCKS + b
                qc[:, b, :] = np.tile(q16[:, g * P : (g + 1) * P], (4, 1))
        coff = c * BLOCKS * P
        kc = np.tile(k16[:, coff : coff + WC], (4, 1))
        in_maps.append({"q": qc, "k": kc})
    return order, W, WC, in_maps


def _host_mask(pos):
    """Exact host fallback for degenerate inputs (f64, blocked)."""
    posf = np.asarray(pos, dtype=np.float64)
    out = np.zeros((N, N), dtype=bool)
    for i0 in range(0, N, 512):
        d2 = ((posf[i0 : i0 + 512, None, :] - posf[None, :, :]) ** 2).sum(-1)
        out[i0 : i0 + 512] = d2 <= R2
    return out


LAST_RESULTS = None  # BassKernelResults of the most recent run (for profiling)


def kernel(pos):
    global LAST_RESULTS
    LAST_RESULTS = None
    prep = _prepare(pos)
    if prep is None:
        return _host_mask(pos)
    order, W, WC, in_maps = prep
    try:
        nc = _build_graph_shared_raw(W, WC)
        res = run_bass_kernel_spmd(nc, in_maps, list(range(NCORES)))
    except Exception as e:  # device failure: fall back to exact host compute
        import sys
        print(f"kernel: device path failed ({type(e).__name__}: {e}); host fallback", file=sys.stderr)
        return _host_mask(pos)
    LAST_RESULTS = res

    full = np.zeros((N, N), dtype=bool)
    for c in range(NCORES):
        o = res.results[c]["out"]                      # [BLOCKS, 128, W] int8
        for b in range(BLOCKS):
            g = c * BLOCKS + b
            start = g * P
            valid = min(W, N - start)
            rows = order[start : start + P]
            cols = order[start : start + valid]
            full[np.ix_(rows, cols)] = o[b][:, :valid] == 1
    full |= full.T
    return full



# revision 3
# speedup vs baseline: 1.1426x; 1.1426x over previous
"""Radius-graph adjacency mask (radius_graph r=3, loop=True) on 8 TRN2 NeuronCores.

Strategy
--------
mask[i, j] = (||p_i - p_j||^2 <= R2)  for pos [8192, 3].

val(i, j) = (R2 + eps) - d2(i, j) is computed as a single small-K matmul:
    val = sum_r q_rows[r, i] * k_rows[r, j]
where the q/k rows hold 3-way bf16 splits of the augmented query/key vectors
(2x, sq terms), so the bf16 TensorE matmul (1 cycle/row) reproduces the fp32
value to ~24-bit accuracy.  PSUM holds val; mask = (val >= 0) via VectorE
is_ge / ScalarE Sign (both engines split the PSUM-read load), written as int8
and DMA'd out.

Sharding: rows data-parallel across 8 cores (1024 query rows each).  Atoms are
z-sorted; in symmetric mode each 128-query block computes only keys at sorted
index >= its own start inside the z-window (all forward |z_i - z_j| <= 3
neighbors) — a W~1024 slab — and the host mirrors the lower triangle.  Each
core holds ONE shared key window [128*8c, 128*8c + WC); block b reads columns
[128b, 128b + W) of it.  The host scatters the slabs into the full mask.
"""

from contextlib import ExitStack

import ml_dtypes
import numpy as np

import concourse.mybir as mybir
from concourse import bacc
from concourse.bass_utils import run_bass_kernel_spmd

N = 8192
R2 = 9.0
RADIUS = 3.0
EPS = 1e-5
NCORES = 8
P = 128
KP = 32                       # padded contraction rows (30 used)
BLOCKS = (N // NCORES) // P   # 8 query blocks of 128 rows per core
BF16 = ml_dtypes.bfloat16

def _bf16_split3(x):
    """Split f64 array into 3 bf16 components summing to ~24-bit accuracy."""
    b0 = x.astype(BF16)
    r1 = x - b0.astype(np.float64)
    b1 = r1.astype(BF16)
    r2 = r1 - b1.astype(np.float64)
    b2 = r2.astype(BF16)
    return b0.astype(np.float64), b1.astype(np.float64), b2.astype(np.float64)


def _build_rows(ps):
    """Build the KP-row augmented query/key matrices (f64 holding bf16 values).

    val = sum_r q_rows[r, i] * k_rows[r, j] = (R2 + EPS) - d2(i, j)
    """
    n = ps.shape[0]
    A = 2.0 * ps.T                      # (3, n) query-side coefficient
    B = ps.T                            # (3, n) key-side
    S = (R2 + EPS) - (ps * ps).sum(1)   # query-side constant term
    T = -(ps * ps).sum(1)               # key-side constant term
    ones = np.ones(n)

    rows_q, rows_k = [], []
    for c in range(3):
        Asp = _bf16_split3(A[c])
        Bsp = _bf16_split3(B[c])
        # all split-product terms above ~2^-32 relative (drop (2,2) only)
        for u, v in [(0, 0), (0, 1), (1, 0), (1, 1), (0, 2), (2, 0), (1, 2), (2, 1)]:
            rows_q.append(Asp[u])
            rows_k.append(Bsp[v])
    for s in _bf16_split3(S):
        rows_q.append(s)
        rows_k.append(ones)
    for t in _bf16_split3(T):
        rows_q.append(ones)
        rows_k.append(t)

    q = np.zeros((KP, n))
    k = np.zeros((KP, n))
    q[: len(rows_q)] = np.stack(rows_q)
    k[: len(rows_k)] = np.stack(rows_k)
    return q, k



def _psum_slot(b, engine_map=True):
    """engine_map: ACT (even b) slots 0/1 (banks 0-3); DVE (odd b) slots 2/3."""
    return (b % 2) * 2 + (b // 2) % 2 if engine_map else b % 4

def _build_graph_shared_raw(W, WC, final_wait=False, k2_sync=True, psum_engine_map=True,
                            last_split=False, swap_parity=False):
    """Raw Block version of the symmetric shared-window graph.

    Manual engine streams + semaphores (no TileContext): saves the Tile
    entry/exit drain + barrier + sem-clear machinery (~4us of exec window).

    Engine roles: sync = q + k-tail DMA; scalar = k-head DMA + Sign x4;
    vector = is_ge x4; tensor = matmuls; gpsimd = output DMAs.

    When NT == 2 the q tensor is packed: each block only needs row groups
    {2b%4, (2b+1)%4}, so even blocks live at partitions 0..63, odd at 64..127,
    slot b//2 — halving the q transfer.
    """
    assert W % 64 == 0 and W <= 2048
    NT = -(-W // 512)
    q_packed = NT == 2
    QS = BLOCKS // 2 if q_packed else BLOCKS

    def eng_of(b):  # which engine thresholds block b
        return "act" if (b % 2 == 0) != swap_parity else "dve"

    nc = bacc.Bacc("TRN2", target_bir_lowering=False)
    q_ext = nc.declare_dram_parameter("q", [P, QS, P], mybir.dt.bfloat16, isOutput=False)
    k_ext = nc.declare_dram_parameter("k", [P, WC], mybir.dt.bfloat16, isOutput=False)
    out_ext = nc.declare_dram_parameter("out", [BLOCKS, P, W], mybir.dt.int8, isOutput=True)

    # count of same-engine thresholds among blocks 0..b inclusive
    def eng_count(b, eng):
        return sum(1 for x in range(b + 1) if eng_of(x) == eng)

    with ExitStack() as ctx:
        qsem = ctx.enter_context(nc.semaphore("qsem"))
        ksem = ctx.enter_context(nc.semaphore("ksem"))
        ksem1b = ctx.enter_context(nc.semaphore("ksem1b"))
        ksem2 = ctx.enter_context(nc.semaphore("ksem2"))
        ksem2b = ctx.enter_context(nc.semaphore("ksem2b"))
        pe_sem = ctx.enter_context(nc.semaphore("pe_sem"))
        act_sem = ctx.enter_context(nc.semaphore("act_sem"))
        dve_sem = ctx.enter_context(nc.semaphore("dve_sem"))
        osem = ctx.enter_context(nc.semaphore("osem"))
        wsem = ctx.enter_context(nc.semaphore("wsem"))
        scratch = ctx.enter_context(nc.sbuf_tensor("scratch", [P, 640], mybir.dt.bfloat16))
        q_sb = ctx.enter_context(nc.sbuf_tensor("q_sb", [P, QS, P], mybir.dt.bfloat16))
        k_sb = ctx.enter_context(nc.sbuf_tensor("k_sb", [P, WC], mybir.dt.bfloat16))
        masks = [
            ctx.enter_context(nc.sbuf_tensor(f"m{i}", [P, 2, W], mybir.dt.int8))
            for i in range(BLOCKS // 2)
        ]
        psums = [
            ctx.enter_context(nc.psum_tensor(f"ps{i}", [P, W], mybir.dt.float32))
            for i in range(4)
        ]

        SPLIT_B = BLOCKS - 1                  # last block: split across engines
        # balance ACT (4 full blocks + H cols) vs DVE (3 full + W-H cols):
        # 5*oA + (4W+H)*eA = 4*oD + (4W-H)*eD with per-op overheads/rates
        _h = (4 * 125 - 5 * 143 + 4 * W * (1.042 - 0.833)) / (0.833 + 1.042)
        HALF = int(max(64, min(W - 64, round(_h / 64) * 64)))

        def _thresh(engine, b, lo=0, hi=None):
            hi = W if hi is None else hi
            slot = _psum_slot(b, psum_engine_map)
            if engine.engine == mybir.EngineType.Activation:
                return engine.activation(
                    masks[b // 2][:, b % 2, lo:hi], psums[slot][:, lo:hi],
                    mybir.ActivationFunctionType.Sign,
                ).then_inc(act_sem, 1)
            return engine.tensor_scalar(
                masks[b // 2][:, b % 2, lo:hi], psums[slot][:, lo:hi],
                0.0, None, mybir.AluOpType.is_ge,
            ).then_inc(dve_sem, 1)

        with nc.Block() as block:

            MID = W + max(64, ((WC - W) // 2) // 64 * 64) if WC > W else WC
            # key pieces: [start, end, sem) — MMs wait per piece on first use.
            # One [0:W] head so block 0's two matmul tiles (distinct PE row
            # groups) become ready together and run concurrently.
            pieces = [(0, W, ksem)]
            if WC > W:
                pieces.append((W, MID, ksem2))
                if MID < WC:
                    pieces.append((MID, WC, ksem2b))

            @block.sync
            def _(sync):
                sync.dma_start(out=q_sb[:], in_=q_ext[:]).then_inc(qsem, 16)
                if WC > W and MID < WC:
                    sync.dma_start(out=k_sb[:, MID:], in_=k_ext[:, MID:]).then_inc(ksem2b, 16)

            @block.scalar
            def _(scalar):
                scalar.dma_start(out=k_sb[:, :W], in_=k_ext[:, :W]).then_inc(ksem, 16)
                for b in range(BLOCKS):
                    if b == SPLIT_B:
                        scalar.wait_ge(pe_sem, b + 1)
                        _thresh(scalar, b, 0, HALF)
                    elif eng_of(b) == "act":
                        scalar.wait_ge(pe_sem, b + 1)
                        _thresh(scalar, b)

            @block.vector
            def _(vector):
                vector.memset(scratch[:], 0).then_inc(wsem, 1)
                for b in range(BLOCKS):
                    if b == SPLIT_B:
                        vector.wait_ge(pe_sem, b + 1)
                        _thresh(vector, b, HALF, W)
                    elif eng_of(b) == "dve":
                        vector.wait_ge(pe_sem, b + 1)
                        _thresh(vector, b)

            @block.tensor
            def _(tensor):
                # HAM warmup: ~3us of dummy matmuls on zeroed scratch while
                # the input DMAs are in flight, so real matmuls run at 2.4 GHz.
                # Results land in ps0 and are overwritten (start=True) later.
                tensor.wait_ge(wsem, 1)
                for w in range(5):
                    g = 2 + w % 2          # groups 2/3: block 0 uses 0/1
                    # psums[3] (its owner b3 shares row groups 2/3 so it
                    # serializes after); per-group DISTINCT banks: concurrent
                    # PE writes to one PSUM bank are a fatal collision
                    wn = 512 if g == 2 else min(448, W - 512)
                    wo = 0 if g == 2 else 512
                    tensor.matmul(
                        psums[3][:, wo : wo + wn],
                        lhsT=scratch[32 * g : 32 * (g + 1), :128],
                        rhs=scratch[32 * g : 32 * (g + 1), 128 : 128 + wn],
                        start=True,
                        stop=True,
                        tile_position=(32 * g, 0),
                    )
                tensor.wait_ge(qsem, 16)
                tensor.wait_ge(ksem, 16)
                waited = {id(ksem)}
                for b in range(BLOCKS):
                    if b >= 4:  # psum slot reuse: wait for block b-4's threshold
                        prev = b - 4
                        if eng_of(prev) == "act":
                            tensor.wait_ge(act_sem, eng_count(prev, "act"))
                        else:
                            tensor.wait_ge(dve_sem, eng_count(prev, "dve"))

                    for t in range(NT):
                        g = (NT * b + t) % 4
                        col = P * b + 512 * t
                        nn = min(512, W - 512 * t)
                        for p0, p1, sem in pieces:
                            if id(sem) not in waited and col + nn > p0 and col < p1:
                                tensor.wait_ge(sem, 16)
                                waited.add(id(sem))
                        mm = tensor.matmul(
                            psums[_psum_slot(b, psum_engine_map)][:, 512 * t : 512 * t + nn],
                            lhsT=q_sb[32 * g : 32 * (g + 1), b // 2 if q_packed else b, :],
                            rhs=k_sb[32 * g : 32 * (g + 1), col : col + nn],
                            start=True,
                            stop=True,
                            tile_position=(32 * g, 0),
                        )
                        if t == NT - 1:
                            mm.then_inc(pe_sem, 1)

            @block.gpsimd
            def _(gpsimd):
                if WC > W:
                    gpsimd.dma_start(out=k_sb[:, W:MID], in_=k_ext[:, W:MID]).then_inc(ksem2, 16)
                last = BLOCKS // 2 - 1
                ndma = 0
                for i in range(last):
                    gpsimd.wait_ge(act_sem, i + 1)
                    gpsimd.wait_ge(dve_sem, i + 1)
                    gpsimd.dma_start(
                        out=out_ext[2 * i : 2 * i + 2, :, :].rearrange("b p w -> p b w"),
                        in_=masks[i][:],
                    ).then_inc(osem, 16)
                    ndma += 1
                if last_split:
                    s0 = act_sem if eng_of(2 * last) == "act" else dve_sem
                    s1 = act_sem if eng_of(2 * last + 1) == "act" else dve_sem
                    gpsimd.wait_ge(s0, last + 1)
                    gpsimd.dma_start(
                        out=out_ext[2 * last : 2 * last + 1, :, :].rearrange("b p w -> p b w"),
                        in_=masks[last][:, :1],
                    ).then_inc(osem, 16)
                    gpsimd.wait_ge(s1, last + 1)
                    gpsimd.dma_start(
                        out=out_ext[2 * last + 1 : 2 * last + 2, :, :].rearrange("b p w -> p b w"),
                        in_=masks[last][:, 1:],
                    ).then_inc(osem, 16)
                    ndma += 2
                else:
                    # block 2*last is a normal single-engine threshold; block
                    # 2*last+1 (SPLIT_B) contributes one inc on EACH engine
                    gpsimd.wait_ge(act_sem, eng_count(BLOCKS - 2, "act") + 1)
                    gpsimd.wait_ge(dve_sem, eng_count(BLOCKS - 2, "dve") + 1)
                    gpsimd.dma_start(
                        out=out_ext[2 * last : 2 * last + 2, :, :].rearrange("b p w -> p b w"),
                        in_=masks[last][:],
                    ).then_inc(osem, 16)
                    ndma += 1
                if final_wait:
                    gpsimd.wait_ge(osem, 16 * ndma)

    nc.compile()
    return nc


def _build_graph_v2(W, WC, warmups=5):
    """Restructured raw-Block graph, tuned from the 22.1us trace of v1.

    Changes vs _build_graph_shared_raw:
    - q gets the SP HWDGE queue alone (no k tail behind it): its completion
      sem (the matmul gate) fires ~1.4us earlier.
    - k is split [0:640) / [640:1280) / [1280:WC) across ACT (x2) and Pool
      queues; matmul tiles wait per-piece on first use.
    - out is [P, BLOCKS, W] (partition-major) so each pair DMA writes 2W
      contiguous bytes per partition (1920B descriptors vs 960B), and the 4
      pair DMAs alternate between the SP and Pool queues.
    - warmup matmuls run on uninitialized scratch with no memset dependency:
      the PE starts its HAM ramp at kernel entry (~2.4us of extra warmup).
    - PSUM tensors are [P, 1024] f32 (2 full banks each): tile halves of one
      block and neighboring slots can never share a bank.
    """
    assert W % 64 == 0 and W <= 1024
    NT = -(-W // 512)
    q_packed = NT == 2
    QS = BLOCKS // 2 if q_packed else BLOCKS

    def eng_of(b):  # which engine thresholds block b
        return "act" if b % 2 == 0 else "dve"

    def eng_count(b, eng):
        return sum(1 for x in range(b + 1) if eng_of(x) == eng)

    nc = bacc.Bacc("TRN2", target_bir_lowering=False)
    q_ext = nc.declare_dram_parameter("q", [P, QS, P], mybir.dt.bfloat16, isOutput=False)
    k_ext = nc.declare_dram_parameter("k", [P, WC], mybir.dt.bfloat16, isOutput=False)
    out_ext = nc.declare_dram_parameter("out", [P, BLOCKS, W], mybir.dt.int8, isOutput=True)

    with ExitStack() as ctx:
        qsem = ctx.enter_context(nc.semaphore("qsem"))
        ksemA = ctx.enter_context(nc.semaphore("ksemA"))
        ksemB = ctx.enter_context(nc.semaphore("ksemB"))
        ksemC = ctx.enter_context(nc.semaphore("ksemC"))
        pe_sem = ctx.enter_context(nc.semaphore("pe_sem"))
        act_sem = ctx.enter_context(nc.semaphore("act_sem"))
        dve_sem = ctx.enter_context(nc.semaphore("dve_sem"))
        osem = ctx.enter_context(nc.semaphore("osem"))
        scratch = ctx.enter_context(nc.sbuf_tensor("scratch", [P, 576], mybir.dt.bfloat16))
        q_sb = ctx.enter_context(nc.sbuf_tensor("q_sb", [P, QS, P], mybir.dt.bfloat16))
        k_sb = ctx.enter_context(nc.sbuf_tensor("k_sb", [P, WC], mybir.dt.bfloat16))
        masks = [
            ctx.enter_context(nc.sbuf_tensor(f"m{i}", [P, 2, W], mybir.dt.int8))
            for i in range(BLOCKS // 2)
        ]
        psums = [
            ctx.enter_context(nc.psum_tensor(f"ps{i}", [P, 1024], mybir.dt.float32))
            for i in range(4)
        ]

        SPLIT_B = BLOCKS - 1
        _h = (4 * 125 - 5 * 143 + 4 * W * (1.042 - 0.833)) / (0.833 + 1.042)
        HALF = int(max(64, min(W - 64, round(_h / 64) * 64)))

        # k pieces: [start, end, sem); ordered by first need in the tensor
        # stream.  A and B go on the ACT HWDGE queue (FIFO: A drains first),
        # C on the Pool SWDGE queue.
        pb1 = min(WC, 640)
        pb2 = min(WC, 1280)
        pieces = [(0, pb1, ksemA)]
        if pb1 < pb2:
            pieces.append((pb1, pb2, ksemB))
        if pb2 < WC:
            pieces.append((pb2, WC, ksemC))

        def _thresh(engine, b, lo=0, hi=None):
            hi = W if hi is None else hi
            slot = _psum_slot(b)
            if engine.engine == mybir.EngineType.Activation:
                return engine.activation(
                    masks[b // 2][:, b % 2, lo:hi], psums[slot][:, lo:hi],
                    mybir.ActivationFunctionType.Sign,
                ).then_inc(act_sem, 1)
            return engine.tensor_scalar(
                masks[b // 2][:, b % 2, lo:hi], psums[slot][:, lo:hi],
                0.0, None, mybir.AluOpType.is_ge,
            ).then_inc(dve_sem, 1)

        def _out_pair(engine, i):
            # mask pair i -> blocks 2i, 2i+1; wait for both thresholds
            ba, bd = 2 * i, 2 * i + 1
            na = eng_count(ba if eng_of(ba) == "act" else bd, "act")
            nd = eng_count(bd if eng_of(bd) == "dve" else ba, "dve")
            if 2 * i + 1 == SPLIT_B:  # split block: one inc on EACH engine
                na, nd = eng_count(BLOCKS - 2, "act") + 1, eng_count(BLOCKS - 2, "dve") + 1
            engine.wait_ge(act_sem, na)
            engine.wait_ge(dve_sem, nd)
            engine.dma_start(
                out=out_ext[:, 2 * i : 2 * i + 2, :], in_=masks[i][:]
            ).then_inc(osem, 16)

        with nc.Block() as block:

            @block.sync
            def _(sync):
                sync.dma_start(out=q_sb[:], in_=q_ext[:]).then_inc(qsem, 16)
                _out_pair(sync, 0)
                _out_pair(sync, 2)

            @block.scalar
            def _(scalar):
                scalar.dma_start(out=k_sb[:, :pb1], in_=k_ext[:, :pb1]).then_inc(ksemA, 16)
                if pb1 < pb2:
                    scalar.dma_start(out=k_sb[:, pb1:pb2], in_=k_ext[:, pb1:pb2]).then_inc(ksemB, 16)
                for b in range(BLOCKS):
                    if b == SPLIT_B:
                        scalar.wait_ge(pe_sem, b + 1)
                        _thresh(scalar, b, 0, HALF)
                    elif eng_of(b) == "act":
                        scalar.wait_ge(pe_sem, b + 1)
                        _thresh(scalar, b)

            @block.vector
            def _(vector):
                for b in range(BLOCKS):
                    if b == SPLIT_B:
                        vector.wait_ge(pe_sem, b + 1)
                        _thresh(vector, b, HALF, W)
                    elif eng_of(b) == "dve":
                        vector.wait_ge(pe_sem, b + 1)
                        _thresh(vector, b)

            @block.tensor
            def _(tensor):
                # HAM warmup on (uninitialized) scratch from kernel entry;
                # psums[3] is overwritten by block 3's start=True matmul later.
                for w in range(warmups):
                    g = 2 + w % 2
                    wn = 448
                    wo = 0 if g == 2 else 512
                    tensor.matmul(
                        psums[3][:, wo : wo + wn],
                        lhsT=scratch[32 * g : 32 * (g + 1), :128],
                        rhs=scratch[32 * g : 32 * (g + 1), 128 : 128 + wn],
                        start=True,
                        stop=True,
                        tile_position=(32 * g, 0),
                    )
                tensor.wait_ge(qsem, 16)
                waited = set()
                for b in range(BLOCKS):
                    if b >= 4:  # psum slot reuse: wait for block b-4's threshold
                        prev = b - 4
                        if eng_of(prev) == "act":
                            tensor.wait_ge(act_sem, eng_count(prev, "act"))
                        else:
                            tensor.wait_ge(dve_sem, eng_count(prev, "dve"))
                    for t in range(NT):
                        g = (NT * b + t) % 4
                        col = P * b + 512 * t
                        nn = min(512, W - 512 * t)
                        for p0, p1, sem in pieces:
                            if id(sem) not in waited and col + nn > p0 and col < p1:
                                tensor.wait_ge(sem, 16)
                                waited.add(id(sem))
                        mm = tensor.matmul(
                            psums[_psum_slot(b)][:, 512 * t : 512 * t + nn],
                            lhsT=q_sb[32 * g : 32 * (g + 1), b // 2 if q_packed else b, :],
                            rhs=k_sb[32 * g : 32 * (g + 1), col : col + nn],
                            start=True,
                            stop=True,
                            tile_position=(32 * g, 0),
                        )
                        if t == NT - 1:
                            mm.then_inc(pe_sem, 1)

            @block.gpsimd
            def _(gpsimd):
                if pb2 < WC:
                    gpsimd.dma_start(out=k_sb[:, pb2:], in_=k_ext[:, pb2:]).then_inc(ksemC, 16)
                _out_pair(gpsimd, 1)
                _out_pair(gpsimd, 3)

    nc.compile()
    return nc


def _prepare(pos):
    """Host prep: pick the sort axis with the tightest symmetric window, build
    per-core in_maps.  Returns None when no axis gives a device-sized window
    (degenerate clustered input) -- caller falls back to host computation."""
    posf = np.asarray(pos, dtype=np.float64)
    nblocks = N // P

    # recenter: d2 is translation-invariant, but smaller |coords| shrink the
    # fp32 cancellation error in sq_i + sq_j - 2 x.y by ~4x
    posf = posf - (posf.min(0) + posf.max(0)) / 2.0

    best = None
    for axis in range(3):
        order = np.argsort(posf[:, axis], kind="stable")
        z = posf[order][:, axis]
        zb = z.reshape(nblocks, P)
        ihi = np.searchsorted(z, zb.max(1) + RADIUS, side="right")
        w_sym = int((ihi - np.arange(nblocks, dtype=np.int64) * P).max())
        if best is None or w_sym < best[0]:
            best = (w_sym, order)
    w_sym, order = best
    if w_sym > 2048:
        return None

    ps = posf[order]
    W = max(512, -(-w_sym // 64) * 64)
    WC = P * (BLOCKS - 1) + W
    q_packed = -(-W // 512) == 2
    qrows, krows = _build_rows(ps)
    q16 = qrows.astype(BF16)
    # pad key tail with far-away dummies (mask always 0 there)
    k16 = np.zeros((KP, N + WC), dtype=BF16)
    k16[:, :N] = krows.astype(BF16)
    k16[KP - 3, N:] = -1e9              # T0 row: val = S_i - 1e9 < 0
    in_maps = []
    for c in range(NCORES):
        if q_packed:
            # block b lives at row groups {2b%4, (2b+1)%4}, slot b//2
            qc = np.zeros((P, BLOCKS // 2, P), dtype=BF16)
            for b in range(BLOCKS):
                g = c * BLOCKS + b
                qb = q16[:, g * P : (g + 1) * P]
                base = 0 if b % 2 == 0 else 64
                qc[base : base + 64, b // 2, :] = np.tile(qb, (2, 1))
        else:
            qc = np.zeros((P, BLOCKS, P), dtype=BF16)
            for b in range(BLOCKS):
                g = c * BLOCKS + b
                qc[:, b, :] = np.tile(q16[:, g * P : (g + 1) * P], (4, 1))
        coff = c * BLOCKS * P
        kc = np.tile(k16[:, coff : coff + WC], (4, 1))
        in_maps.append({"q": qc, "k": kc})
    return order, W, WC, in_maps


def _host_mask(pos):
    """Exact host fallback for degenerate inputs (f64, blocked)."""
    posf = np.asarray(pos, dtype=np.float64)
    out = np.zeros((N, N), dtype=bool)
    for i0 in range(0, N, 512):
        d2 = ((posf[i0 : i0 + 512, None, :] - posf[None, :, :]) ** 2).sum(-1)
        out[i0 : i0 + 512] = d2 <= R2
    return out


LAST_RESULTS = None  # BassKernelResults of the most recent run (for profiling)


def kernel(pos):
    global LAST_RESULTS
    LAST_RESULTS = None
    prep = _prepare(pos)
    if prep is None:
        return _host_mask(pos)
    order, W, WC, in_maps = prep
    try:
        nc = _build_graph_v2(W, WC)
        res = run_bass_kernel_spmd(nc, in_maps, list(range(NCORES)))
    except Exception as e:  # device failure: fall back to exact host compute
        import sys
        print(f"kernel: device path failed ({type(e).__name__}: {e}); host fallback", file=sys.stderr)
        return _host_mask(pos)
    LAST_RESULTS = res

    full = np.zeros((N, N), dtype=bool)
    for c in range(NCORES):
        o = res.results[c]["out"]                      # [128, BLOCKS, W] int8
        for b in range(BLOCKS):
            g = c * BLOCKS + b
            start = g * P
            valid = min(W, N - start)
            rows = order[start : start + P]
            cols = order[start : start + valid]
            full[np.ix_(rows, cols)] = o[:, b, :valid] == 1
    full |= full.T
    return full



# revision 8
# speedup vs baseline: 1.2031x; 1.0530x over previous
"""Radius-graph adjacency mask (radius_graph r=3, loop=True) on 8 TRN2 NeuronCores.

Strategy
--------
mask[i, j] = (||p_i - p_j||^2 <= R2)  for pos [8192, 3].

val(i, j) = (R2 + eps) - d2(i, j) is computed as a single small-K matmul:
    val = sum_r q_rows[r, i] * k_rows[r, j]
where the q/k rows hold 3-way bf16 splits of the augmented query/key vectors
(2x, sq terms), so the bf16 TensorE matmul (1 cycle/row) reproduces the fp32
value to ~24-bit accuracy.  PSUM holds val; mask = (val >= 0) via VectorE
is_ge / ScalarE Sign (both engines split the PSUM-read load), written as int8
and DMA'd out.

Sharding: rows data-parallel across 8 cores (1024 query rows each).  Atoms are
z-sorted; in symmetric mode each 128-query block computes only keys at sorted
index >= its own start inside the z-window (all forward |z_i - z_j| <= 3
neighbors) — a W~1024 slab — and the host mirrors the lower triangle.  Each
core holds ONE shared key window [128*8c, 128*8c + WC); block b reads columns
[128b, 128b + W) of it.  The host scatters the slabs into the full mask.
"""

from contextlib import ExitStack

import ml_dtypes
import numpy as np

import concourse.mybir as mybir
from concourse import bacc
from concourse.bass_utils import run_bass_kernel_spmd

N = 8192
R2 = 9.0
RADIUS = 3.0
EPS = 1e-5
NCORES = 8
P = 128
KP = 32                       # padded contraction rows (30 used)
BLOCKS = (N // NCORES) // P   # 8 query blocks of 128 rows per core
BF16 = ml_dtypes.bfloat16

def _bf16_split3(x):
    """Split f64 array into 3 bf16 components summing to ~24-bit accuracy."""
    b0 = x.astype(BF16)
    r1 = x - b0.astype(np.float64)
    b1 = r1.astype(BF16)
    r2 = r1 - b1.astype(np.float64)
    b2 = r2.astype(BF16)
    return b0.astype(np.float64), b1.astype(np.float64), b2.astype(np.float64)


def _build_rows(ps):
    """Build the KP-row augmented query/key matrices (f64 holding bf16 values).

    val = sum_r q_rows[r, i] * k_rows[r, j] = (R2 + EPS) - d2(i, j)
    """
    n = ps.shape[0]
    A = 2.0 * ps.T                      # (3, n) query-side coefficient
    B = ps.T                            # (3, n) key-side
    S = (R2 + EPS) - (ps * ps).sum(1)   # query-side constant term
    T = -(ps * ps).sum(1)               # key-side constant term
    ones = np.ones(n)

    rows_q, rows_k = [], []
    for c in range(3):
        Asp = _bf16_split3(A[c])
        Bsp = _bf16_split3(B[c])
        # all split-product terms above ~2^-32 relative (drop (2,2) only)
        for u, v in [(0, 0), (0, 1), (1, 0), (1, 1), (0, 2), (2, 0), (1, 2), (2, 1)]:
            rows_q.append(Asp[u])
            rows_k.append(Bsp[v])
    for s in _bf16_split3(S):
        rows_q.append(s)
        rows_k.append(ones)
    for t in _bf16_split3(T):
        rows_q.append(ones)
        rows_k.append(t)

    q = np.zeros((KP, n))
    k = np.zeros((KP, n))
    q[: len(rows_q)] = np.stack(rows_q)
    k[: len(rows_k)] = np.stack(rows_k)
    return q, k



def _psum_slot(b, engine_map=True):
    """engine_map: ACT (even b) slots 0/1 (banks 0-3); DVE (odd b) slots 2/3."""
    return (b % 2) * 2 + (b // 2) % 2 if engine_map else b % 4

def _build_graph_shared_raw(W, WC, final_wait=False, k2_sync=True, psum_engine_map=True,
                            last_split=False, swap_parity=False):
    """Raw Block version of the symmetric shared-window graph.

    Manual engine streams + semaphores (no TileContext): saves the Tile
    entry/exit drain + barrier + sem-clear machinery (~4us of exec window).

    Engine roles: sync = q + k-tail DMA; scalar = k-head DMA + Sign x4;
    vector = is_ge x4; tensor = matmuls; gpsimd = output DMAs.

    When NT == 2 the q tensor is packed: each block only needs row groups
    {2b%4, (2b+1)%4}, so even blocks live at partitions 0..63, odd at 64..127,
    slot b//2 — halving the q transfer.
    """
    assert W % 64 == 0 and W <= 2048
    NT = -(-W // 512)
    q_packed = NT == 2
    QS = BLOCKS // 2 if q_packed else BLOCKS

    def eng_of(b):  # which engine thresholds block b
        return "act" if (b % 2 == 0) != swap_parity else "dve"

    nc = bacc.Bacc("TRN2", target_bir_lowering=False)
    q_ext = nc.declare_dram_parameter("q", [P, QS, P], mybir.dt.bfloat16, isOutput=False)
    k_ext = nc.declare_dram_parameter("k", [P, WC], mybir.dt.bfloat16, isOutput=False)
    out_ext = nc.declare_dram_parameter("out", [BLOCKS, P, W], mybir.dt.int8, isOutput=True)

    # count of same-engine thresholds among blocks 0..b inclusive
    def eng_count(b, eng):
        return sum(1 for x in range(b + 1) if eng_of(x) == eng)

    with ExitStack() as ctx:
        qsem = ctx.enter_context(nc.semaphore("qsem"))
        ksem = ctx.enter_context(nc.semaphore("ksem"))
        ksem1b = ctx.enter_context(nc.semaphore("ksem1b"))
        ksem2 = ctx.enter_context(nc.semaphore("ksem2"))
        ksem2b = ctx.enter_context(nc.semaphore("ksem2b"))
        pe_sem = ctx.enter_context(nc.semaphore("pe_sem"))
        act_sem = ctx.enter_context(nc.semaphore("act_sem"))
        dve_sem = ctx.enter_context(nc.semaphore("dve_sem"))
        osem = ctx.enter_context(nc.semaphore("osem"))
        wsem = ctx.enter_context(nc.semaphore("wsem"))
        scratch = ctx.enter_context(nc.sbuf_tensor("scratch", [P, 640], mybir.dt.bfloat16))
        q_sb = ctx.enter_context(nc.sbuf_tensor("q_sb", [P, QS, P], mybir.dt.bfloat16))
        k_sb = ctx.enter_context(nc.sbuf_tensor("k_sb", [P, WC], mybir.dt.bfloat16))
        masks = [
            ctx.enter_context(nc.sbuf_tensor(f"m{i}", [P, 2, W], mybir.dt.int8))
            for i in range(BLOCKS // 2)
        ]
        psums = [
            ctx.enter_context(nc.psum_tensor(f"ps{i}", [P, W], mybir.dt.float32))
            for i in range(4)
        ]

        SPLIT_B = BLOCKS - 1                  # last block: split across engines
        # balance ACT (4 full blocks + H cols) vs DVE (3 full + W-H cols):
        # 5*oA + (4W+H)*eA = 4*oD + (4W-H)*eD with per-op overheads/rates
        _h = (4 * 125 - 5 * 143 + 4 * W * (1.042 - 0.833)) / (0.833 + 1.042)
        HALF = int(max(64, min(W - 64, round(_h / 64) * 64)))

        def _thresh(engine, b, lo=0, hi=None):
            hi = W if hi is None else hi
            slot = _psum_slot(b, psum_engine_map)
            if engine.engine == mybir.EngineType.Activation:
                return engine.activation(
                    masks[b // 2][:, b % 2, lo:hi], psums[slot][:, lo:hi],
                    mybir.ActivationFunctionType.Sign,
                ).then_inc(act_sem, 1)
            return engine.tensor_scalar(
                masks[b // 2][:, b % 2, lo:hi], psums[slot][:, lo:hi],
                0.0, None, mybir.AluOpType.is_ge,
            ).then_inc(dve_sem, 1)

        with nc.Block() as block:

            MID = W + max(64, ((WC - W) // 2) // 64 * 64) if WC > W else WC
            # key pieces: [start, end, sem) — MMs wait per piece on first use.
            # One [0:W] head so block 0's two matmul tiles (distinct PE row
            # groups) become ready together and run concurrently.
            pieces = [(0, W, ksem)]
            if WC > W:
                pieces.append((W, MID, ksem2))
                if MID < WC:
                    pieces.append((MID, WC, ksem2b))

            @block.sync
            def _(sync):
                sync.dma_start(out=q_sb[:], in_=q_ext[:]).then_inc(qsem, 16)
                if WC > W and MID < WC:
                    sync.dma_start(out=k_sb[:, MID:], in_=k_ext[:, MID:]).then_inc(ksem2b, 16)

            @block.scalar
            def _(scalar):
                scalar.dma_start(out=k_sb[:, :W], in_=k_ext[:, :W]).then_inc(ksem, 16)
                for b in range(BLOCKS):
                    if b == SPLIT_B:
                        scalar.wait_ge(pe_sem, b + 1)
                        _thresh(scalar, b, 0, HALF)
                    elif eng_of(b) == "act":
                        scalar.wait_ge(pe_sem, b + 1)
                        _thresh(scalar, b)

            @block.vector
            def _(vector):
                vector.memset(scratch[:], 0).then_inc(wsem, 1)
                for b in range(BLOCKS):
                    if b == SPLIT_B:
                        vector.wait_ge(pe_sem, b + 1)
                        _thresh(vector, b, HALF, W)
                    elif eng_of(b) == "dve":
                        vector.wait_ge(pe_sem, b + 1)
                        _thresh(vector, b)

            @block.tensor
            def _(tensor):
                # HAM warmup: ~3us of dummy matmuls on zeroed scratch while
                # the input DMAs are in flight, so real matmuls run at 2.4 GHz.
                # Results land in ps0 and are overwritten (start=True) later.
                tensor.wait_ge(wsem, 1)
                for w in range(5):
                    g = 2 + w % 2          # groups 2/3: block 0 uses 0/1
                    # psums[3] (its owner b3 shares row groups 2/3 so it
                    # serializes after); per-group DISTINCT banks: concurrent
                    # PE writes to one PSUM bank are a fatal collision
                    wn = 512 if g == 2 else min(448, W - 512)
                    wo = 0 if g == 2 else 512
                    tensor.matmul(
                        psums[3][:, wo : wo + wn],
                        lhsT=scratch[32 * g : 32 * (g + 1), :128],
                        rhs=scratch[32 * g : 32 * (g + 1), 128 : 128 + wn],
                        start=True,
                        stop=True,
                        tile_position=(32 * g, 0),
                    )
                tensor.wait_ge(qsem, 16)
                tensor.wait_ge(ksem, 16)
                waited = {id(ksem)}
                for b in range(BLOCKS):
                    if b >= 4:  # psum slot reuse: wait for block b-4's threshold
                        prev = b - 4
                        if eng_of(prev) == "act":
                            tensor.wait_ge(act_sem, eng_count(prev, "act"))
                        else:
                            tensor.wait_ge(dve_sem, eng_count(prev, "dve"))

                    for t in range(NT):
                        g = (NT * b + t) % 4
                        col = P * b + 512 * t
                        nn = min(512, W - 512 * t)
                        for p0, p1, sem in pieces:
                            if id(sem) not in waited and col + nn > p0 and col < p1:
                                tensor.wait_ge(sem, 16)
                                waited.add(id(sem))
                        mm = tensor.matmul(
                            psums[_psum_slot(b, psum_engine_map)][:, 512 * t : 512 * t + nn],
                            lhsT=q_sb[32 * g : 32 * (g + 1), b // 2 if q_packed else b, :],
                            rhs=k_sb[32 * g : 32 * (g + 1), col : col + nn],
                            start=True,
                            stop=True,
                            tile_position=(32 * g, 0),
                        )
                        if t == NT - 1:
                            mm.then_inc(pe_sem, 1)

            @block.gpsimd
            def _(gpsimd):
                if WC > W:
                    gpsimd.dma_start(out=k_sb[:, W:MID], in_=k_ext[:, W:MID]).then_inc(ksem2, 16)
                last = BLOCKS // 2 - 1
                ndma = 0
                for i in range(last):
                    gpsimd.wait_ge(act_sem, i + 1)
                    gpsimd.wait_ge(dve_sem, i + 1)
                    gpsimd.dma_start(
                        out=out_ext[2 * i : 2 * i + 2, :, :].rearrange("b p w -> p b w"),
                        in_=masks[i][:],
                    ).then_inc(osem, 16)
                    ndma += 1
                if last_split:
                    s0 = act_sem if eng_of(2 * last) == "act" else dve_sem
                    s1 = act_sem if eng_of(2 * last + 1) == "act" else dve_sem
                    gpsimd.wait_ge(s0, last + 1)
                    gpsimd.dma_start(
                        out=out_ext[2 * last : 2 * last + 1, :, :].rearrange("b p w -> p b w"),
                        in_=masks[last][:, :1],
                    ).then_inc(osem, 16)
                    gpsimd.wait_ge(s1, last + 1)
                    gpsimd.dma_start(
                        out=out_ext[2 * last + 1 : 2 * last + 2, :, :].rearrange("b p w -> p b w"),
                        in_=masks[last][:, 1:],
                    ).then_inc(osem, 16)
                    ndma += 2
                else:
                    # block 2*last is a normal single-engine threshold; block
                    # 2*last+1 (SPLIT_B) contributes one inc on EACH engine
                    gpsimd.wait_ge(act_sem, eng_count(BLOCKS - 2, "act") + 1)
                    gpsimd.wait_ge(dve_sem, eng_count(BLOCKS - 2, "dve") + 1)
                    gpsimd.dma_start(
                        out=out_ext[2 * last : 2 * last + 2, :, :].rearrange("b p w -> p b w"),
                        in_=masks[last][:],
                    ).then_inc(osem, 16)
                    ndma += 1
                if final_wait:
                    gpsimd.wait_ge(osem, 16 * ndma)

    nc.compile()
    return nc


def _build_graph_v2(W, WC, warmups=5):
    """Restructured raw-Block graph, tuned from the 22.1us trace of v1.

    Changes vs _build_graph_shared_raw:
    - q gets the SP HWDGE queue alone (no k tail behind it): its completion
      sem (the matmul gate) fires ~1.4us earlier.
    - k is split [0:640) / [640:1280) / [1280:WC) across ACT (x2) and Pool
      queues; matmul tiles wait per-piece on first use.
    - out is [P, BLOCKS, W] (partition-major) so each pair DMA writes 2W
      contiguous bytes per partition (1920B descriptors vs 960B), and the 4
      pair DMAs alternate between the SP and Pool queues.
    - warmup matmuls run on uninitialized scratch with no memset dependency:
      the PE starts its HAM ramp at kernel entry (~2.4us of extra warmup).
    - PSUM tensors are [P, 1024] f32 (2 full banks each): tile halves of one
      block and neighboring slots can never share a bank.
    """
    assert W % 64 == 0 and W <= 1024
    NT = -(-W // 512)
    q_packed = NT == 2
    QS = BLOCKS // 2 if q_packed else BLOCKS

    def eng_of(b):  # which engine thresholds block b
        return "act" if b % 2 == 0 else "dve"

    def eng_count(b, eng):
        return sum(1 for x in range(b + 1) if eng_of(x) == eng)

    nc = bacc.Bacc("TRN2", target_bir_lowering=False)
    q_ext = nc.declare_dram_parameter("q", [P, QS, P], mybir.dt.bfloat16, isOutput=False)
    k_ext = nc.declare_dram_parameter("k", [P, WC], mybir.dt.bfloat16, isOutput=False)
    out_ext = nc.declare_dram_parameter("out", [P, BLOCKS, W], mybir.dt.int8, isOutput=True)

    with ExitStack() as ctx:
        qsem = ctx.enter_context(nc.semaphore("qsem"))
        ksemA = ctx.enter_context(nc.semaphore("ksemA"))
        ksemB = ctx.enter_context(nc.semaphore("ksemB"))
        ksemC = ctx.enter_context(nc.semaphore("ksemC"))
        ksemD = ctx.enter_context(nc.semaphore("ksemD"))
        pe_sem = ctx.enter_context(nc.semaphore("pe_sem"))
        act_sem = ctx.enter_context(nc.semaphore("act_sem"))
        dve_sem = ctx.enter_context(nc.semaphore("dve_sem"))
        osem = ctx.enter_context(nc.semaphore("osem"))
        scratch = ctx.enter_context(nc.sbuf_tensor("scratch", [P, 576], mybir.dt.bfloat16))
        q_sb = ctx.enter_context(nc.sbuf_tensor("q_sb", [P, QS, P], mybir.dt.bfloat16))
        k_sb = ctx.enter_context(nc.sbuf_tensor("k_sb", [P, WC], mybir.dt.bfloat16))
        masks = [
            ctx.enter_context(nc.sbuf_tensor(f"m{i}", [P, 2, W], mybir.dt.int8))
            for i in range(BLOCKS // 2)
        ]
        psums = [
            ctx.enter_context(nc.psum_tensor(f"ps{i}", [P, 1024], mybir.dt.float32))
            for i in range(4)
        ]

        SPLIT_B = BLOCKS - 1
        _h = (4 * 125 - 5 * 143 + 4 * W * (1.042 - 0.833)) / (0.833 + 1.042)
        HALF = int(max(64, min(W - 64, round(_h / 64) * 64)))

        # k pieces: [start, end, sem); split across all three DMA queues so
        # the head pieces' completion sems fire as early as possible.
        # A -> ACT, B -> SP (behind q), C -> Pool, D -> ACT (behind A).
        pb1 = min(WC, 512)
        pb2 = min(WC, 1088)
        pb3 = min(WC, 1600)
        pieces = [(0, pb1, ksemA)]
        if pb1 < pb2:
            pieces.append((pb1, pb2, ksemB))
        if pb2 < pb3:
            pieces.append((pb2, pb3, ksemC))
        if pb3 < WC:
            pieces.append((pb3, WC, ksemD))

        def _thresh(engine, b, lo=0, hi=None):
            hi = W if hi is None else hi
            slot = _psum_slot(b)
            if engine.engine == mybir.EngineType.Activation:
                return engine.activation(
                    masks[b // 2][:, b % 2, lo:hi], psums[slot][:, lo:hi],
                    mybir.ActivationFunctionType.Sign,
                ).then_inc(act_sem, 1)
            return engine.tensor_scalar(
                masks[b // 2][:, b % 2, lo:hi], psums[slot][:, lo:hi],
                0.0, None, mybir.AluOpType.is_ge,
            ).then_inc(dve_sem, 1)

        def _out_pair(engine, i):
            # mask pair i -> blocks 2i, 2i+1; wait for both thresholds
            ba, bd = 2 * i, 2 * i + 1
            na = eng_count(ba if eng_of(ba) == "act" else bd, "act")
            nd = eng_count(bd if eng_of(bd) == "dve" else ba, "dve")
            if 2 * i + 1 == SPLIT_B:  # split block: one inc on EACH engine
                na, nd = eng_count(BLOCKS - 2, "act") + 1, eng_count(BLOCKS - 2, "dve") + 1
            engine.wait_ge(act_sem, na)
            engine.wait_ge(dve_sem, nd)
            engine.dma_start(
                out=out_ext[:, 2 * i : 2 * i + 2, :], in_=masks[i][:]
            ).then_inc(osem, 16)

        with nc.Block() as block:

            @block.sync
            def _(sync):
                sync.dma_start(out=q_sb[:], in_=q_ext[:]).then_inc(qsem, 16)
                if pb1 < pb2:
                    sync.dma_start(out=k_sb[:, pb1:pb2], in_=k_ext[:, pb1:pb2]).then_inc(ksemB, 16)
                _out_pair(sync, 0)
                _out_pair(sync, 2)
                _out_pair(sync, 3)

            @block.scalar
            def _(scalar):
                scalar.dma_start(out=k_sb[:, :pb1], in_=k_ext[:, :pb1]).then_inc(ksemA, 16)
                if pb3 < WC:
                    scalar.dma_start(out=k_sb[:, pb3:], in_=k_ext[:, pb3:]).then_inc(ksemD, 16)
                for b in range(BLOCKS):
                    if b == SPLIT_B:
                        scalar.wait_ge(pe_sem, b + 1)
                        _thresh(scalar, b, 0, HALF)
                    elif eng_of(b) == "act":
                        scalar.wait_ge(pe_sem, b + 1)
                        _thresh(scalar, b)

            @block.vector
            def _(vector):
                for b in range(BLOCKS):
                    if b == SPLIT_B:
                        vector.wait_ge(pe_sem, b + 1)
                        _thresh(vector, b, HALF, W)
                    elif eng_of(b) == "dve":
                        vector.wait_ge(pe_sem, b + 1)
                        _thresh(vector, b)

            @block.tensor
            def _(tensor):
                # HAM warmup on (uninitialized) scratch from kernel entry;
                # psums[3] is overwritten by block 3's start=True matmul later.
                # Single group (g=2) so warmups serialize: each ~0.5us, filling
                # the window until the input DMA sems fire (~3us).
                for w in range(warmups):
                    tensor.matmul(
                        psums[3][:, :448],
                        lhsT=scratch[64:96, :128],
                        rhs=scratch[64:96, 128:576],
                        start=True,
                        stop=True,
                        tile_position=(64, 0),
                    )
                tensor.wait_ge(qsem, 16)
                waited = set()
                for b in range(BLOCKS):
                    if b >= 4:  # psum slot reuse: wait for block b-4's threshold
                        prev = b - 4
                        if eng_of(prev) == "act":
                            tensor.wait_ge(act_sem, eng_count(prev, "act"))
                        else:
                            tensor.wait_ge(dve_sem, eng_count(prev, "dve"))
                    for t in range(NT):
                        g = (NT * b + t) % 4
                        col = P * b + 512 * t
                        nn = min(512, W - 512 * t)
                        for p0, p1, sem in pieces:
                            if id(sem) not in waited and col + nn > p0 and col < p1:
                                tensor.wait_ge(sem, 16)
                                waited.add(id(sem))
                        mm = tensor.matmul(
                            psums[_psum_slot(b)][:, 512 * t : 512 * t + nn],
                            lhsT=q_sb[32 * g : 32 * (g + 1), b // 2 if q_packed else b, :],
                            rhs=k_sb[32 * g : 32 * (g + 1), col : col + nn],
                            start=True,
                            stop=True,
                            tile_position=(32 * g, 0),
                        )
                        if t == NT - 1:
                            mm.then_inc(pe_sem, 1)

            @block.gpsimd
            def _(gpsimd):
                if pb2 < pb3:
                    gpsimd.dma_start(out=k_sb[:, pb2:pb3], in_=k_ext[:, pb2:pb3]).then_inc(ksemC, 16)
                _out_pair(gpsimd, 1)

    nc.compile()
    return nc


def _prepare(pos):
    """Host prep: pick the sort axis with the tightest symmetric window, build
    per-core in_maps.  Returns None when no axis gives a device-sized window
    (degenerate clustered input) -- caller falls back to host computation."""
    posf = np.asarray(pos, dtype=np.float64)
    nblocks = N // P

    # recenter: d2 is translation-invariant, but smaller |coords| shrink the
    # fp32 cancellation error in sq_i + sq_j - 2 x.y by ~4x
    posf = posf - (posf.min(0) + posf.max(0)) / 2.0

    best = None
    for axis in range(3):
        order = np.argsort(posf[:, axis], kind="stable")
        z = posf[order][:, axis]
        zb = z.reshape(nblocks, P)
        ihi = np.searchsorted(z, zb.max(1) + RADIUS, side="right")
        w_sym = int((ihi - np.arange(nblocks, dtype=np.int64) * P).max())
        if best is None or w_sym < best[0]:
            best = (w_sym, order)
    w_sym, order = best
    if w_sym > 2048:
        return None

    ps = posf[order]
    W = max(512, -(-w_sym // 64) * 64)
    WC = P * (BLOCKS - 1) + W
    q_packed = -(-W // 512) == 2
    qrows, krows = _build_rows(ps)
    q16 = qrows.astype(BF16)
    # pad key tail with far-away dummies (mask always 0 there)
    k16 = np.zeros((KP, N + WC), dtype=BF16)
    k16[:, :N] = krows.astype(BF16)
    k16[KP - 3, N:] = -1e9              # T0 row: val = S_i - 1e9 < 0
    in_maps = []
    for c in range(NCORES):
        if q_packed:
            # block b lives at row groups {2b%4, (2b+1)%4}, slot b//2
            qc = np.zeros((P, BLOCKS // 2, P), dtype=BF16)
            for b in range(BLOCKS):
                g = c * BLOCKS + b
                qb = q16[:, g * P : (g + 1) * P]
                base = 0 if b % 2 == 0 else 64
                qc[base : base + 64, b // 2, :] = np.tile(qb, (2, 1))
        else:
            qc = np.zeros((P, BLOCKS, P), dtype=BF16)
            for b in range(BLOCKS):
                g = c * BLOCKS + b
                qc[:, b, :] = np.tile(q16[:, g * P : (g + 1) * P], (4, 1))
        coff = c * BLOCKS * P
        kc = np.tile(k16[:, coff : coff + WC], (4, 1))
        in_maps.append({"q": qc, "k": kc})
    return order, W, WC, in_maps


def _host_mask(pos):
    """Exact host fallback for degenerate inputs (f64, blocked)."""
    posf = np.asarray(pos, dtype=np.float64)
    out = np.zeros((N, N), dtype=bool)
    for i0 in range(0, N, 512):
        d2 = ((posf[i0 : i0 + 512, None, :] - posf[None, :, :]) ** 2).sum(-1)
        out[i0 : i0 + 512] = d2 <= R2
    return out


LAST_RESULTS = None  # BassKernelResults of the most recent run (for profiling)


def kernel(pos):
    global LAST_RESULTS
    LAST_RESULTS = None
    prep = _prepare(pos)
    if prep is None:
        return _host_mask(pos)
    order, W, WC, in_maps = prep
    try:
        nc = _build_graph_v2(W, WC)
        res = run_bass_kernel_spmd(nc, in_maps, list(range(NCORES)))
    except Exception as e:  # device failure: fall back to exact host compute
        import sys
        print(f"kernel: device path failed ({type(e).__name__}: {e}); host fallback", file=sys.stderr)
        return _host_mask(pos)
    LAST_RESULTS = res

    full = np.zeros((N, N), dtype=bool)
    for c in range(NCORES):
        o = res.results[c]["out"]                      # [128, BLOCKS, W] int8
        for b in range(BLOCKS):
            g = c * BLOCKS + b
            start = g * P
            valid = min(W, N - start)
            rows = order[start : start + P]
            cols = order[start : start + valid]
            full[np.ix_(rows, cols)] = o[:, b, :valid] == 1
    full |= full.T
    return full



# revision 10
# speedup vs baseline: 1.2187x; 1.0130x over previous
"""Radius-graph adjacency mask (radius_graph r=3, loop=True) on 8 TRN2 NeuronCores.

Strategy
--------
mask[i, j] = (||p_i - p_j||^2 <= R2)  for pos [8192, 3].

val(i, j) = (R2 + eps) - d2(i, j) is computed as a single small-K matmul:
    val = sum_r q_rows[r, i] * k_rows[r, j]
where the q/k rows hold 3-way bf16 splits of the augmented query/key vectors
(2x, sq terms), so the bf16 TensorE matmul (1 cycle/row) reproduces the fp32
value to ~24-bit accuracy.  PSUM holds val; mask = (val >= 0) via VectorE
is_ge / ScalarE Sign (both engines split the PSUM-read load), written as int8
and DMA'd out.

Sharding: rows data-parallel across 8 cores (1024 query rows each).  Atoms are
z-sorted; in symmetric mode each 128-query block computes only keys at sorted
index >= its own start inside the z-window (all forward |z_i - z_j| <= 3
neighbors) — a W~1024 slab — and the host mirrors the lower triangle.  Each
core holds ONE shared key window [128*8c, 128*8c + WC); block b reads columns
[128b, 128b + W) of it.  The host scatters the slabs into the full mask.
"""

from contextlib import ExitStack

import ml_dtypes
import numpy as np

import concourse.mybir as mybir
from concourse import bacc
from concourse.bass_utils import run_bass_kernel_spmd

N = 8192
R2 = 9.0
RADIUS = 3.0
EPS = 1e-5
NCORES = 8
P = 128
KP = 32                       # padded contraction rows (30 used)
BLOCKS = (N // NCORES) // P   # 8 query blocks of 128 rows per core
BF16 = ml_dtypes.bfloat16

def _bf16_split3(x):
    """Split f64 array into 3 bf16 components summing to ~24-bit accuracy."""
    b0 = x.astype(BF16)
    r1 = x - b0.astype(np.float64)
    b1 = r1.astype(BF16)
    r2 = r1 - b1.astype(np.float64)
    b2 = r2.astype(BF16)
    return b0.astype(np.float64), b1.astype(np.float64), b2.astype(np.float64)


def _build_rows(ps):
    """Build the KP-row augmented query/key matrices (f64 holding bf16 values).

    val = sum_r q_rows[r, i] * k_rows[r, j] = (R2 + EPS) - d2(i, j)
    """
    n = ps.shape[0]
    A = 2.0 * ps.T                      # (3, n) query-side coefficient
    B = ps.T                            # (3, n) key-side
    S = (R2 + EPS) - (ps * ps).sum(1)   # query-side constant term
    T = -(ps * ps).sum(1)               # key-side constant term
    ones = np.ones(n)

    rows_q, rows_k = [], []
    for c in range(3):
        Asp = _bf16_split3(A[c])
        Bsp = _bf16_split3(B[c])
        # all split-product terms above ~2^-32 relative (drop (2,2) only)
        for u, v in [(0, 0), (0, 1), (1, 0), (1, 1), (0, 2), (2, 0), (1, 2), (2, 1)]:
            rows_q.append(Asp[u])
            rows_k.append(Bsp[v])
    for s in _bf16_split3(S):
        rows_q.append(s)
        rows_k.append(ones)
    for t in _bf16_split3(T):
        rows_q.append(ones)
        rows_k.append(t)

    q = np.zeros((KP, n))
    k = np.zeros((KP, n))
    q[: len(rows_q)] = np.stack(rows_q)
    k[: len(rows_k)] = np.stack(rows_k)
    return q, k



def _psum_slot(b, engine_map=True):
    """engine_map: ACT (even b) slots 0/1 (banks 0-3); DVE (odd b) slots 2/3."""
    return (b % 2) * 2 + (b // 2) % 2 if engine_map else b % 4

def _build_graph_shared_raw(W, WC, final_wait=False, k2_sync=True, psum_engine_map=True,
                            last_split=False, swap_parity=False):
    """Raw Block version of the symmetric shared-window graph.

    Manual engine streams + semaphores (no TileContext): saves the Tile
    entry/exit drain + barrier + sem-clear machinery (~4us of exec window).

    Engine roles: sync = q + k-tail DMA; scalar = k-head DMA + Sign x4;
    vector = is_ge x4; tensor = matmuls; gpsimd = output DMAs.

    When NT == 2 the q tensor is packed: each block only needs row groups
    {2b%4, (2b+1)%4}, so even blocks live at partitions 0..63, odd at 64..127,
    slot b//2 — halving the q transfer.
    """
    assert W % 64 == 0 and W <= 2048
    NT = -(-W // 512)
    q_packed = NT == 2
    QS = BLOCKS // 2 if q_packed else BLOCKS

    def eng_of(b):  # which engine thresholds block b
        return "act" if (b % 2 == 0) != swap_parity else "dve"

    nc = bacc.Bacc("TRN2", target_bir_lowering=False)
    q_ext = nc.declare_dram_parameter("q", [P, QS, P], mybir.dt.bfloat16, isOutput=False)
    k_ext = nc.declare_dram_parameter("k", [P, WC], mybir.dt.bfloat16, isOutput=False)
    out_ext = nc.declare_dram_parameter("out", [BLOCKS, P, W], mybir.dt.int8, isOutput=True)

    # count of same-engine thresholds among blocks 0..b inclusive
    def eng_count(b, eng):
        return sum(1 for x in range(b + 1) if eng_of(x) == eng)

    with ExitStack() as ctx:
        qsem = ctx.enter_context(nc.semaphore("qsem"))
        ksem = ctx.enter_context(nc.semaphore("ksem"))
        ksem1b = ctx.enter_context(nc.semaphore("ksem1b"))
        ksem2 = ctx.enter_context(nc.semaphore("ksem2"))
        ksem2b = ctx.enter_context(nc.semaphore("ksem2b"))
        pe_sem = ctx.enter_context(nc.semaphore("pe_sem"))
        act_sem = ctx.enter_context(nc.semaphore("act_sem"))
        dve_sem = ctx.enter_context(nc.semaphore("dve_sem"))
        osem = ctx.enter_context(nc.semaphore("osem"))
        wsem = ctx.enter_context(nc.semaphore("wsem"))
        scratch = ctx.enter_context(nc.sbuf_tensor("scratch", [P, 640], mybir.dt.bfloat16))
        q_sb = ctx.enter_context(nc.sbuf_tensor("q_sb", [P, QS, P], mybir.dt.bfloat16))
        k_sb = ctx.enter_context(nc.sbuf_tensor("k_sb", [P, WC], mybir.dt.bfloat16))
        masks = [
            ctx.enter_context(nc.sbuf_tensor(f"m{i}", [P, 2, W], mybir.dt.int8))
            for i in range(BLOCKS // 2)
        ]
        psums = [
            ctx.enter_context(nc.psum_tensor(f"ps{i}", [P, W], mybir.dt.float32))
            for i in range(4)
        ]

        SPLIT_B = BLOCKS - 1                  # last block: split across engines
        # balance ACT (4 full blocks + H cols) vs DVE (3 full + W-H cols):
        # 5*oA + (4W+H)*eA = 4*oD + (4W-H)*eD with per-op overheads/rates
        _h = (4 * 125 - 5 * 143 + 4 * W * (1.042 - 0.833)) / (0.833 + 1.042)
        HALF = int(max(64, min(W - 64, round(_h / 64) * 64)))

        def _thresh(engine, b, lo=0, hi=None):
            hi = W if hi is None else hi
            slot = _psum_slot(b, psum_engine_map)
            if engine.engine == mybir.EngineType.Activation:
                return engine.activation(
                    masks[b // 2][:, b % 2, lo:hi], psums[slot][:, lo:hi],
                    mybir.ActivationFunctionType.Sign,
                ).then_inc(act_sem, 1)
            return engine.tensor_scalar(
                masks[b // 2][:, b % 2, lo:hi], psums[slot][:, lo:hi],
                0.0, None, mybir.AluOpType.is_ge,
            ).then_inc(dve_sem, 1)

        with nc.Block() as block:

            MID = W + max(64, ((WC - W) // 2) // 64 * 64) if WC > W else WC
            # key pieces: [start, end, sem) — MMs wait per piece on first use.
            # One [0:W] head so block 0's two matmul tiles (distinct PE row
            # groups) become ready together and run concurrently.
            pieces = [(0, W, ksem)]
            if WC > W:
                pieces.append((W, MID, ksem2))
                if MID < WC:
                    pieces.append((MID, WC, ksem2b))

            @block.sync
            def _(sync):
                sync.dma_start(out=q_sb[:], in_=q_ext[:]).then_inc(qsem, 16)
                if WC > W and MID < WC:
                    sync.dma_start(out=k_sb[:, MID:], in_=k_ext[:, MID:]).then_inc(ksem2b, 16)

            @block.scalar
            def _(scalar):
                scalar.dma_start(out=k_sb[:, :W], in_=k_ext[:, :W]).then_inc(ksem, 16)
                for b in range(BLOCKS):
                    if b == SPLIT_B:
                        scalar.wait_ge(pe_sem, b + 1)
                        _thresh(scalar, b, 0, HALF)
                    elif eng_of(b) == "act":
                        scalar.wait_ge(pe_sem, b + 1)
                        _thresh(scalar, b)

            @block.vector
            def _(vector):
                vector.memset(scratch[:], 0).then_inc(wsem, 1)
                for b in range(BLOCKS):
                    if b == SPLIT_B:
                        vector.wait_ge(pe_sem, b + 1)
                        _thresh(vector, b, HALF, W)
                    elif eng_of(b) == "dve":
                        vector.wait_ge(pe_sem, b + 1)
                        _thresh(vector, b)

            @block.tensor
            def _(tensor):
                # HAM warmup: ~3us of dummy matmuls on zeroed scratch while
                # the input DMAs are in flight, so real matmuls run at 2.4 GHz.
                # Results land in ps0 and are overwritten (start=True) later.
                tensor.wait_ge(wsem, 1)
                for w in range(5):
                    g = 2 + w % 2          # groups 2/3: block 0 uses 0/1
                    # psums[3] (its owner b3 shares row groups 2/3 so it
                    # serializes after); per-group DISTINCT banks: concurrent
                    # PE writes to one PSUM bank are a fatal collision
                    wn = 512 if g == 2 else min(448, W - 512)
                    wo = 0 if g == 2 else 512
                    tensor.matmul(
                        psums[3][:, wo : wo + wn],
                        lhsT=scratch[32 * g : 32 * (g + 1), :128],
                        rhs=scratch[32 * g : 32 * (g + 1), 128 : 128 + wn],
                        start=True,
                        stop=True,
                        tile_position=(32 * g, 0),
                    )
                tensor.wait_ge(qsem, 16)
                tensor.wait_ge(ksem, 16)
                waited = {id(ksem)}
                for b in range(BLOCKS):
                    if b >= 4:  # psum slot reuse: wait for block b-4's threshold
                        prev = b - 4
                        if eng_of(prev) == "act":
                            tensor.wait_ge(act_sem, eng_count(prev, "act"))
                        else:
                            tensor.wait_ge(dve_sem, eng_count(prev, "dve"))

                    for t in range(NT):
                        g = (NT * b + t) % 4
                        col = P * b + 512 * t
                        nn = min(512, W - 512 * t)
                        for p0, p1, sem in pieces:
                            if id(sem) not in waited and col + nn > p0 and col < p1:
                                tensor.wait_ge(sem, 16)
                                waited.add(id(sem))
                        mm = tensor.matmul(
                            psums[_psum_slot(b, psum_engine_map)][:, 512 * t : 512 * t + nn],
                            lhsT=q_sb[32 * g : 32 * (g + 1), b // 2 if q_packed else b, :],
                            rhs=k_sb[32 * g : 32 * (g + 1), col : col + nn],
                            start=True,
                            stop=True,
                            tile_position=(32 * g, 0),
                        )
                        if t == NT - 1:
                            mm.then_inc(pe_sem, 1)

            @block.gpsimd
            def _(gpsimd):
                if WC > W:
                    gpsimd.dma_start(out=k_sb[:, W:MID], in_=k_ext[:, W:MID]).then_inc(ksem2, 16)
                last = BLOCKS // 2 - 1
                ndma = 0
                for i in range(last):
                    gpsimd.wait_ge(act_sem, i + 1)
                    gpsimd.wait_ge(dve_sem, i + 1)
                    gpsimd.dma_start(
                        out=out_ext[2 * i : 2 * i + 2, :, :].rearrange("b p w -> p b w"),
                        in_=masks[i][:],
                    ).then_inc(osem, 16)
                    ndma += 1
                if last_split:
                    s0 = act_sem if eng_of(2 * last) == "act" else dve_sem
                    s1 = act_sem if eng_of(2 * last + 1) == "act" else dve_sem
                    gpsimd.wait_ge(s0, last + 1)
                    gpsimd.dma_start(
                        out=out_ext[2 * last : 2 * last + 1, :, :].rearrange("b p w -> p b w"),
                        in_=masks[last][:, :1],
                    ).then_inc(osem, 16)
                    gpsimd.wait_ge(s1, last + 1)
                    gpsimd.dma_start(
                        out=out_ext[2 * last + 1 : 2 * last + 2, :, :].rearrange("b p w -> p b w"),
                        in_=masks[last][:, 1:],
                    ).then_inc(osem, 16)
                    ndma += 2
                else:
                    # block 2*last is a normal single-engine threshold; block
                    # 2*last+1 (SPLIT_B) contributes one inc on EACH engine
                    gpsimd.wait_ge(act_sem, eng_count(BLOCKS - 2, "act") + 1)
                    gpsimd.wait_ge(dve_sem, eng_count(BLOCKS - 2, "dve") + 1)
                    gpsimd.dma_start(
                        out=out_ext[2 * last : 2 * last + 2, :, :].rearrange("b p w -> p b w"),
                        in_=masks[last][:],
                    ).then_inc(osem, 16)
                    ndma += 1
                if final_wait:
                    gpsimd.wait_ge(osem, 16 * ndma)

    nc.compile()
    return nc


def _build_graph_v2(W, WC, warmups=5):
    """Restructured raw-Block graph, tuned from the 22.1us trace of v1.

    Changes vs _build_graph_shared_raw:
    - q gets the SP HWDGE queue alone (no k tail behind it): its completion
      sem (the matmul gate) fires ~1.4us earlier.
    - k is split [0:640) / [640:1280) / [1280:WC) across ACT (x2) and Pool
      queues; matmul tiles wait per-piece on first use.
    - out is [P, BLOCKS, W] (partition-major) so each pair DMA writes 2W
      contiguous bytes per partition (1920B descriptors vs 960B), and the 4
      pair DMAs alternate between the SP and Pool queues.
    - warmup matmuls run on uninitialized scratch with no memset dependency:
      the PE starts its HAM ramp at kernel entry (~2.4us of extra warmup).
    - PSUM tensors are [P, 1024] f32 (2 full banks each): tile halves of one
      block and neighboring slots can never share a bank.
    """
    assert W % 64 == 0 and W <= 1024
    NT = -(-W // 512)
    q_packed = NT == 2
    QS = BLOCKS // 2 if q_packed else BLOCKS

    def eng_of(b):  # which engine thresholds block b
        return "act" if b % 2 == 0 else "dve"

    def eng_count(b, eng):
        return sum(1 for x in range(b + 1) if eng_of(x) == eng)

    nc = bacc.Bacc("TRN2", target_bir_lowering=False)
    q_ext = nc.declare_dram_parameter("q", [P, QS, P], mybir.dt.bfloat16, isOutput=False)
    k_ext = nc.declare_dram_parameter("k", [P, WC], mybir.dt.bfloat16, isOutput=False)
    out_ext = nc.declare_dram_parameter("out", [P, BLOCKS, W], mybir.dt.int8, isOutput=True)

    with ExitStack() as ctx:
        qsem = ctx.enter_context(nc.semaphore("qsem"))
        ksemA = ctx.enter_context(nc.semaphore("ksemA"))
        ksemB = ctx.enter_context(nc.semaphore("ksemB"))
        ksemC = ctx.enter_context(nc.semaphore("ksemC"))
        ksemD = ctx.enter_context(nc.semaphore("ksemD"))
        pe_sem = ctx.enter_context(nc.semaphore("pe_sem"))
        act_sem = ctx.enter_context(nc.semaphore("act_sem"))
        dve_sem = ctx.enter_context(nc.semaphore("dve_sem"))
        osem = ctx.enter_context(nc.semaphore("osem"))
        scratch = ctx.enter_context(nc.sbuf_tensor("scratch", [P, 576], mybir.dt.bfloat16))
        q_sb = ctx.enter_context(nc.sbuf_tensor("q_sb", [P, QS, P], mybir.dt.bfloat16))
        k_sb = ctx.enter_context(nc.sbuf_tensor("k_sb", [P, WC], mybir.dt.bfloat16))
        masks = [
            ctx.enter_context(nc.sbuf_tensor(f"m{i}", [P, 2, W], mybir.dt.int8))
            for i in range(BLOCKS // 2)
        ]
        psums = [
            ctx.enter_context(nc.psum_tensor(f"ps{i}", [P, 1024], mybir.dt.float32))
            for i in range(4)
        ]

        SPLIT_B = BLOCKS - 1
        _h = (4 * 125 - 5 * 143 + 4 * W * (1.042 - 0.833)) / (0.833 + 1.042)
        HALF = int(max(64, min(W - 64, round(_h / 64) * 64)))

        # k pieces: [start, end, sem); split across all three DMA queues so
        # the head pieces' completion sems fire as early as possible.
        # A -> ACT, B -> SP (behind q), C -> Pool, D -> ACT (behind A).
        pb1 = min(WC, 512)
        pb2 = min(WC, 1088)
        pb3 = min(WC, 1600)
        pieces = [(0, pb1, ksemA)]
        if pb1 < pb2:
            pieces.append((pb1, pb2, ksemB))
        if pb2 < pb3:
            pieces.append((pb2, pb3, ksemC))
        if pb3 < WC:
            pieces.append((pb3, WC, ksemD))

        def _thresh(engine, b, lo=0, hi=None):
            hi = W if hi is None else hi
            slot = _psum_slot(b)
            if engine.engine == mybir.EngineType.Activation:
                return engine.activation(
                    masks[b // 2][:, b % 2, lo:hi], psums[slot][:, lo:hi],
                    mybir.ActivationFunctionType.Sign,
                ).then_inc(act_sem, 1)
            return engine.tensor_scalar(
                masks[b // 2][:, b % 2, lo:hi], psums[slot][:, lo:hi],
                0.0, None, mybir.AluOpType.is_ge,
            ).then_inc(dve_sem, 1)

        def _out_pair(engine, i):
            # mask pair i -> blocks 2i, 2i+1; wait for both thresholds
            ba, bd = 2 * i, 2 * i + 1
            na = eng_count(ba if eng_of(ba) == "act" else bd, "act")
            nd = eng_count(bd if eng_of(bd) == "dve" else ba, "dve")
            if 2 * i + 1 == SPLIT_B:  # split block: one inc on EACH engine
                na, nd = eng_count(BLOCKS - 2, "act") + 1, eng_count(BLOCKS - 2, "dve") + 1
            engine.wait_ge(act_sem, na)
            engine.wait_ge(dve_sem, nd)
            engine.dma_start(
                out=out_ext[:, 2 * i : 2 * i + 2, :], in_=masks[i][:]
            ).then_inc(osem, 16)

        with nc.Block() as block:

            @block.sync
            def _(sync):
                sync.dma_start(out=q_sb[:], in_=q_ext[:]).then_inc(qsem, 16)
                if pb1 < pb2:
                    sync.dma_start(out=k_sb[:, pb1:pb2], in_=k_ext[:, pb1:pb2]).then_inc(ksemB, 16)
                _out_pair(sync, 0)
                _out_pair(sync, 2)
                _out_pair(sync, 3)

            @block.scalar
            def _(scalar):
                scalar.dma_start(out=k_sb[:, :pb1], in_=k_ext[:, :pb1]).then_inc(ksemA, 16)
                if pb3 < WC:
                    scalar.dma_start(out=k_sb[:, pb3:], in_=k_ext[:, pb3:]).then_inc(ksemD, 16)
                for b in range(BLOCKS):
                    if b == SPLIT_B:
                        scalar.wait_ge(pe_sem, b + 1)
                        _thresh(scalar, b, 0, HALF)
                    elif eng_of(b) == "act":
                        scalar.wait_ge(pe_sem, b + 1)
                        _thresh(scalar, b)

            @block.vector
            def _(vector):
                for b in range(BLOCKS):
                    if b == SPLIT_B:
                        vector.wait_ge(pe_sem, b + 1)
                        _thresh(vector, b, HALF, W)
                    elif eng_of(b) == "dve":
                        vector.wait_ge(pe_sem, b + 1)
                        _thresh(vector, b)

            @block.tensor
            def _(tensor):
                # HAM warmup on (uninitialized) scratch from kernel entry;
                # psums[3] is overwritten by block 3's start=True matmul later.
                # Single group (g=2) so warmups serialize: each ~0.5us, filling
                # the window until the input DMA sems fire (~3us).
                for w in range(warmups):
                    tensor.matmul(
                        psums[3][:, :448],
                        lhsT=scratch[64:96, :128],
                        rhs=scratch[64:96, 128:576],
                        start=True,
                        stop=True,
                        tile_position=(64, 0),
                    )
                tensor.wait_ge(qsem, 16)
                waited = set()
                for b in range(BLOCKS):
                    if b >= 4:  # psum slot reuse: wait for block b-4's threshold
                        prev = b - 4
                        if eng_of(prev) == "act":
                            tensor.wait_ge(act_sem, eng_count(prev, "act"))
                        else:
                            tensor.wait_ge(dve_sem, eng_count(prev, "dve"))
                    for t in range(NT):
                        g = (NT * b + t) % 4
                        col = P * b + 512 * t
                        nn = min(512, W - 512 * t)
                        for p0, p1, sem in pieces:
                            if id(sem) not in waited and col + nn > p0 and col < p1:
                                tensor.wait_ge(sem, 16)
                                waited.add(id(sem))
                        mm = tensor.matmul(
                            psums[_psum_slot(b)][:, 512 * t : 512 * t + nn],
                            lhsT=q_sb[32 * g : 32 * (g + 1), b // 2 if q_packed else b, :],
                            rhs=k_sb[32 * g : 32 * (g + 1), col : col + nn],
                            start=True,
                            stop=True,
                            tile_position=(32 * g, 0),
                        )
                        if t == NT - 1:
                            mm.then_inc(pe_sem, 1)

            @block.gpsimd
            def _(gpsimd):
                if pb2 < pb3:
                    gpsimd.dma_start(out=k_sb[:, pb2:pb3], in_=k_ext[:, pb2:pb3]).then_inc(ksemC, 16)
                _out_pair(gpsimd, 1)

    nc.compile()
    return nc


def _assign_thresholds(Ws):
    """Exhaustive engine assignment of the 8 slot thresholds to ACT/DVE
    minimizing the longer engine's total cost.  Returns list of 'act'/'dve'."""
    costs_act = [(w + 352) / 1.2 for w in Ws]
    costs_dve = [w / 0.96 + 190 for w in Ws]
    best = None
    for m in range(256):
        a = sum(costs_act[s] for s in range(8) if m >> s & 1)
        d = sum(costs_dve[s] for s in range(8) if not m >> s & 1)
        key = (max(a, d), d)
        if best is None or key < best[0]:
            best = (key, m)
    m = best[1]
    return ["act" if m >> s & 1 else "dve" for s in range(8)]


def _build_graph_gather(Ws, warmups=5):
    """Per-slot gathered-candidate graph: slot s computes a [128, Ws[s]]
    mask tile against its own gathered key columns (host-side bbox gather,
    widths ascending).  Same engine/DMA structure as _build_graph_v2 but with
    ragged widths, per-slot k DMAs round-robined over the 3 queues, and
    threshold engine assignment balanced over the actual widths."""
    assert len(Ws) == BLOCKS
    assert all(w % 64 == 0 and 64 <= w <= 1024 for w in Ws)
    koff = np.concatenate([[0], np.cumsum(Ws)]).astype(int)
    SW = int(koff[-1])
    NT = [-(-w // 512) for w in Ws]
    tiles = []  # (slot, t, col0, nn, group, qslot)
    seq = 0
    for s in range(BLOCKS):
        for t in range(NT[s]):
            nn = min(512, Ws[s] - 512 * t)
            tiles.append((s, t, int(koff[s]) + 512 * t, nn, seq % 4, seq // 4))
            seq += 1
    QS = -(-seq // 4)
    eng = _assign_thresholds(Ws)
    # engine-local threshold order = slot ascending
    def eng_idx(s):  # 1-based count of same-engine thresholds through slot s
        return sum(1 for x in range(s + 1) if eng[x] == eng[s])

    pw = [Ws[2 * i] + Ws[2 * i + 1] for i in range(BLOCKS // 2)]
    poff = [int(koff[2 * i]) for i in range(BLOCKS // 2)]

    nc = bacc.Bacc("TRN2", target_bir_lowering=False)
    q_ext = nc.declare_dram_parameter("q", [P, QS, P], mybir.dt.bfloat16, isOutput=False)
    k_ext = nc.declare_dram_parameter("k", [P, SW], mybir.dt.bfloat16, isOutput=False)
    out_ext = nc.declare_dram_parameter("out", [P, SW], mybir.dt.int8, isOutput=True)

    with ExitStack() as ctx:
        qsem = ctx.enter_context(nc.semaphore("qsem"))
        ksems = [ctx.enter_context(nc.semaphore(f"ksem{s}")) for s in range(BLOCKS)]
        pe_sem = ctx.enter_context(nc.semaphore("pe_sem"))
        act_sem = ctx.enter_context(nc.semaphore("act_sem"))
        dve_sem = ctx.enter_context(nc.semaphore("dve_sem"))
        osem = ctx.enter_context(nc.semaphore("osem"))
        scratch = ctx.enter_context(nc.sbuf_tensor("scratch", [P, 576], mybir.dt.bfloat16))
        q_sb = ctx.enter_context(nc.sbuf_tensor("q_sb", [P, QS, P], mybir.dt.bfloat16))
        k_sb = ctx.enter_context(nc.sbuf_tensor("k_sb", [P, SW], mybir.dt.bfloat16))
        masks = [
            ctx.enter_context(nc.sbuf_tensor(f"m{i}", [P, pw[i]], mybir.dt.int8))
            for i in range(BLOCKS // 2)
        ]
        psums = [
            ctx.enter_context(nc.psum_tensor(f"ps{i}", [P, 1024], mybir.dt.float32))
            for i in range(4)
        ]

        def _thresh(engine, s):
            slot = _psum_slot(s)
            i, lo = s // 2, 0 if s % 2 == 0 else Ws[s - 1]
            if engine.engine == mybir.EngineType.Activation:
                return engine.activation(
                    masks[i][:, lo : lo + Ws[s]], psums[slot][:, : Ws[s]],
                    mybir.ActivationFunctionType.Sign,
                ).then_inc(act_sem, 1)
            return engine.tensor_scalar(
                masks[i][:, lo : lo + Ws[s]], psums[slot][:, : Ws[s]],
                0.0, None, mybir.AluOpType.is_ge,
            ).then_inc(dve_sem, 1)

        def _out_pair(engine, i):
            na = max([eng_idx(b) for b in (2 * i, 2 * i + 1) if eng[b] == "act"], default=0)
            nd = max([eng_idx(b) for b in (2 * i, 2 * i + 1) if eng[b] == "dve"], default=0)
            if na:
                engine.wait_ge(act_sem, na)
            if nd:
                engine.wait_ge(dve_sem, nd)
            engine.dma_start(
                out=out_ext[:, poff[i] : poff[i] + pw[i]], in_=masks[i][:]
            ).then_inc(osem, 16)

        with nc.Block() as block:

            @block.sync
            def _(sync):
                sync.dma_start(out=q_sb[:], in_=q_ext[:]).then_inc(qsem, 16)
                for s in (0, 3, 6):
                    sync.dma_start(
                        out=k_sb[:, koff[s] : koff[s + 1]],
                        in_=k_ext[:, koff[s] : koff[s + 1]],
                    ).then_inc(ksems[s], 16)
                _out_pair(sync, 0)
                _out_pair(sync, 2)
                _out_pair(sync, 3)

            @block.scalar
            def _(scalar):
                for s in (1, 4, 7):
                    scalar.dma_start(
                        out=k_sb[:, koff[s] : koff[s + 1]],
                        in_=k_ext[:, koff[s] : koff[s + 1]],
                    ).then_inc(ksems[s], 16)
                for s in range(BLOCKS):
                    if eng[s] == "act":
                        scalar.wait_ge(pe_sem, s + 1)
                        _thresh(scalar, s)

            @block.vector
            def _(vector):
                for s in range(BLOCKS):
                    if eng[s] == "dve":
                        vector.wait_ge(pe_sem, s + 1)
                        _thresh(vector, s)

            @block.tensor
            def _(tensor):
                for w in range(warmups):
                    tensor.matmul(
                        psums[3][:, :448],
                        lhsT=scratch[64:96, :128],
                        rhs=scratch[64:96, 128:576],
                        start=True,
                        stop=True,
                        tile_position=(64, 0),
                    )
                tensor.wait_ge(qsem, 16)
                waited = set()
                for s, t, col0, nn, g, qslot in tiles:
                    if t == 0 and s >= 4:  # psum slot reuse
                        prev = s - 4
                        if eng[prev] == "act":
                            tensor.wait_ge(act_sem, eng_idx(prev))
                        else:
                            tensor.wait_ge(dve_sem, eng_idx(prev))
                    if s not in waited:
                        tensor.wait_ge(ksems[s], 16)
                        waited.add(s)
                    mm = tensor.matmul(
                        psums[_psum_slot(s)][:, 512 * t : 512 * t + nn],
                        lhsT=q_sb[32 * g : 32 * (g + 1), qslot, :],
                        rhs=k_sb[32 * g : 32 * (g + 1), col0 : col0 + nn],
                        start=True,
                        stop=True,
                        tile_position=(32 * g, 0),
                    )
                    if t == NT[s] - 1:
                        mm.then_inc(pe_sem, 1)

            @block.gpsimd
            def _(gpsimd):
                for s in (2, 5):
                    gpsimd.dma_start(
                        out=k_sb[:, koff[s] : koff[s + 1]],
                        in_=k_ext[:, koff[s] : koff[s + 1]],
                    ).then_inc(ksems[s], 16)
                _out_pair(gpsimd, 1)

    nc.compile()
    return nc


def _prepare_gather(pos):
    """Host prep for the gathered-candidate graph.

    Returns (assign, cands, Ws, in_maps) where assign[c][s] = global block id
    for core c slot s, cands[g] = candidate positions (in sorted order) for
    block g, Ws = 8 slot widths.  None -> fall back to the window path."""
    posf = np.asarray(pos, dtype=np.float64)
    nblocks = N // P
    posf = posf - (posf.min(0) + posf.max(0)) / 2.0

    best = None
    for axis in range(3):
        o = np.argsort(posf[:, axis], kind="stable")
        z = posf[o][:, axis]
        ihi = np.searchsorted(z, z.reshape(nblocks, P).max(1) + RADIUS, side="right")
        w = int((ihi - np.arange(nblocks, dtype=np.int64) * P).max())
        if best is None or w < best[0]:
            best = (w, o)
    _, order = best
    ps = posf[order]

    cands, wid = [], []
    for g in range(nblocks):
        blk = ps[g * P : (g + 1) * P]
        lo, hi = blk.min(0) - RADIUS, blk.max(0) + RADIUS
        tail = ps[g * P :]
        m = ((tail >= lo) & (tail <= hi)).all(1)
        cands.append(np.nonzero(m)[0] + g * P)
        wid.append(len(cands[-1]))
    wid = np.array(wid)

    blocks_sorted = np.argsort(wid, kind="stable")
    assign = blocks_sorted.reshape(BLOCKS, NCORES).T  # [core, slot]
    Ws = [int(-(-wid[blocks_sorted[8 * s : 8 * s + 8]].max() // 64) * 64)
          for s in range(BLOCKS)]
    if any(w > 1024 for w in Ws):
        return None
    koff = np.concatenate([[0], np.cumsum(Ws)]).astype(int)
    SW = int(koff[-1])
    NT = [-(-w // 512) for w in Ws]
    nq = sum(NT)
    QS = -(-nq // 4)

    qrows, krows = _build_rows(ps)
    q16 = qrows.astype(BF16)
    k16 = krows.astype(BF16)

    in_maps = []
    for c in range(NCORES):
        kc = np.zeros((P, SW), dtype=BF16)
        qc = np.zeros((P, QS, P), dtype=BF16)
        seq = 0
        for s in range(BLOCKS):
            g = assign[c][s]
            cnd = cands[g]
            kcols = np.tile(k16[:, cnd], (4, 1))          # [128, w]
            kc[:, koff[s] : koff[s] + len(cnd)] = kcols
            # pad cols: all rows 0 except T0 row (27) = -1e9 -> val < 0
            kc[KP - 3 :: 32, koff[s] + len(cnd) : koff[s + 1]] = np.float32(-1e9)
            qb = q16[:, g * P : (g + 1) * P]
            for t in range(NT[s]):
                gq = seq % 4
                qc[32 * gq : 32 * (gq + 1), seq // 4, :] = qb
                seq += 1
        in_maps.append({"q": qc, "k": kc})
    return assign, cands, Ws, order, in_maps


def _prepare(pos):
    """Host prep: pick the sort axis with the tightest symmetric window, build
    per-core in_maps.  Returns None when no axis gives a device-sized window
    (degenerate clustered input) -- caller falls back to host computation."""
    posf = np.asarray(pos, dtype=np.float64)
    nblocks = N // P

    # recenter: d2 is translation-invariant, but smaller |coords| shrink the
    # fp32 cancellation error in sq_i + sq_j - 2 x.y by ~4x
    posf = posf - (posf.min(0) + posf.max(0)) / 2.0

    best = None
    for axis in range(3):
        order = np.argsort(posf[:, axis], kind="stable")
        z = posf[order][:, axis]
        zb = z.reshape(nblocks, P)
        ihi = np.searchsorted(z, zb.max(1) + RADIUS, side="right")
        w_sym = int((ihi - np.arange(nblocks, dtype=np.int64) * P).max())
        if best is None or w_sym < best[0]:
            best = (w_sym, order)
    w_sym, order = best
    if w_sym > 2048:
        return None

    ps = posf[order]
    W = max(512, -(-w_sym // 64) * 64)
    WC = P * (BLOCKS - 1) + W
    q_packed = -(-W // 512) == 2
    qrows, krows = _build_rows(ps)
    q16 = qrows.astype(BF16)
    # pad key tail with far-away dummies (mask always 0 there)
    k16 = np.zeros((KP, N + WC), dtype=BF16)
    k16[:, :N] = krows.astype(BF16)
    k16[KP - 3, N:] = -1e9              # T0 row: val = S_i - 1e9 < 0
    in_maps = []
    for c in range(NCORES):
        if q_packed:
            # block b lives at row groups {2b%4, (2b+1)%4}, slot b//2
            qc = np.zeros((P, BLOCKS // 2, P), dtype=BF16)
            for b in range(BLOCKS):
                g = c * BLOCKS + b
                qb = q16[:, g * P : (g + 1) * P]
                base = 0 if b % 2 == 0 else 64
                qc[base : base + 64, b // 2, :] = np.tile(qb, (2, 1))
        else:
            qc = np.zeros((P, BLOCKS, P), dtype=BF16)
            for b in range(BLOCKS):
                g = c * BLOCKS + b
                qc[:, b, :] = np.tile(q16[:, g * P : (g + 1) * P], (4, 1))
        coff = c * BLOCKS * P
        kc = np.tile(k16[:, coff : coff + WC], (4, 1))
        in_maps.append({"q": qc, "k": kc})
    return order, W, WC, in_maps


def _host_mask(pos):
    """Exact host fallback for degenerate inputs (f64, blocked)."""
    posf = np.asarray(pos, dtype=np.float64)
    out = np.zeros((N, N), dtype=bool)
    for i0 in range(0, N, 512):
        d2 = ((posf[i0 : i0 + 512, None, :] - posf[None, :, :]) ** 2).sum(-1)
        out[i0 : i0 + 512] = d2 <= R2
    return out


LAST_RESULTS = None  # BassKernelResults of the most recent run (for profiling)


def _kernel_gather(pos):
    prep = _prepare_gather(pos)
    if prep is None:
        return None
    assign, cands, Ws, order, in_maps = prep
    nc = _build_graph_gather(Ws)
    res = run_bass_kernel_spmd(nc, in_maps, list(range(NCORES)))
    koff = np.concatenate([[0], np.cumsum(Ws)]).astype(int)

    full = np.zeros((N, N), dtype=bool)
    for c in range(NCORES):
        o = res.results[c]["out"]                      # [128, SW] int8
        for s in range(BLOCKS):
            g = assign[c][s]
            cnd = cands[g]
            rows = order[g * P : (g + 1) * P]
            cols = order[cnd]
            full[np.ix_(rows, cols)] = o[:, koff[s] : koff[s] + len(cnd)] == 1
    full |= full.T
    return full, res


def _kernel_window(pos):
    prep = _prepare(pos)
    if prep is None:
        return None
    order, W, WC, in_maps = prep
    nc = _build_graph_v2(W, WC)
    res = run_bass_kernel_spmd(nc, in_maps, list(range(NCORES)))

    full = np.zeros((N, N), dtype=bool)
    for c in range(NCORES):
        o = res.results[c]["out"]                      # [128, BLOCKS, W] int8
        for b in range(BLOCKS):
            g = c * BLOCKS + b
            start = g * P
            valid = min(W, N - start)
            rows = order[start : start + P]
            cols = order[start : start + valid]
            full[np.ix_(rows, cols)] = o[:, b, :valid] == 1
    full |= full.T
    return full, res


def kernel(pos):
    global LAST_RESULTS
    LAST_RESULTS = None
    import sys
    try:
        r = _kernel_gather(pos)
    except Exception as e:
        print(f"kernel: gather path failed ({type(e).__name__}: {e}); window fallback", file=sys.stderr)
        r = None
    if r is None:
        try:
            r = _kernel_window(pos)
        except Exception as e:
            print(f"kernel: window path failed ({type(e).__name__}: {e}); host fallback", file=sys.stderr)
            r = None
    if r is None:
        return _host_mask(pos)
    full, res = r
    LAST_RESULTS = res
    return full

